# revision 10
# baseline (speedup 1.0000x reference)
"""Differential multi-head attention (DiffMHA) Trainium2 kernel.

Full-input contract: kernel(**inputs) takes the unsharded numpy inputs and
returns the full [2, 2048, 1024] f32 output. Internally the work is sharded
across 8 NeuronCores: data-parallel over the batch (B=2) and tensor-parallel
over heads (2 value heads / 4 score half-heads per core), with Wq/Wk/Wv
column-sharded and Wo row-sharded. Each core produces a full-width partial
Y contribution; the host sums the 4 head-group partials per batch element
(the "all-reduce" of Megatron row-parallelism, done on host for free).

Per-core pipeline (all matmuls bf16 x bf16 -> f32 PSUM):
  1. inputs f32 -> SBUF, cast bf16, DMA-transpose to [c, t] layout
  2. qT/kT = Wq^T X^T ([d, t]), V = X Wv (natural [n, e])
  3. per 512-wide t-chunk: scores^T = kT^T qT per half-head (causal-skipped),
     exp on ScalarE, rowsums via ones-matmul, O^T = V^T A^T accumulation,
     deferred softmax division + differential-lambda combine on VectorE,
     RMS norm via ones-matmul of squares + partition-broadcast scale
  4. Y += O_scaled^T^T @ Wo_scaled  (rms_weight * (1-lambda_init) is folded
     into Wo rows on-chip), DMA out.
"""

import math
from contextlib import ExitStack

import numpy as np

import concourse.bacc as bacc
import concourse.mybir as mybir
import concourse.tile as tile
from concourse.bass_utils import run_bass_kernel_spmd
from concourse.masks import make_upper_triangular

F32 = mybir.dt.float32
BF16 = mybir.dt.bfloat16
AF = mybir.ActivationFunctionType

T = 2048
C = 1024
DH = 128          # value-head dim
HHD = 64          # score half-head dim
NCH = 4           # t-chunks of 512
TCH = 512
NT = 16           # n-tiles of 128
LAMBDA_INIT = 0.8 - 0.6 * math.exp(-0.3 * 12)
SCALE = 1.0 / math.sqrt(HHD)


def build_nc():
    nc = bacc.Bacc("TRN2", target_bir_lowering=False, debug=False,
                   enable_asserts=False)
    q_d = nc.dram_tensor("query", [T, C], F32, kind="ExternalInput").ap()
    k_d = nc.dram_tensor("key", [T, C], F32, kind="ExternalInput").ap()
    v_d = nc.dram_tensor("value", [T, C], F32, kind="ExternalInput").ap()
    wq_d = nc.dram_tensor("wq", [C, 256], F32, kind="ExternalInput").ap()
    wk_d = nc.dram_tensor("wk", [C, 256], F32, kind="ExternalInput").ap()
    wv_d = nc.dram_tensor("wv", [C, 256], F32, kind="ExternalInput").ap()
    wo_d = nc.dram_tensor("wo", [256, C], F32, kind="ExternalInput").ap()
    lam_d = {nm: nc.dram_tensor(nm, [1, HHD], F32, kind="ExternalInput").ap()
             for nm in ("lq1", "lq2", "lk1", "lk2")}
    rms_d = nc.dram_tensor("rmsw", [128, 1], F32, kind="ExternalInput").ap()
    y_d = nc.dram_tensor("y", [T, C], F32, kind="ExternalOutput").ap()

    with tile.TileContext(nc) as tc, ExitStack() as ctx:
        const = ctx.enter_context(tc.tile_pool(name="const", bufs=1))
        wpool = ctx.enter_context(tc.tile_pool(name="wpool", bufs=1))
        stage = ctx.enter_context(tc.tile_pool(name="stage", bufs=3))
        bstage = ctx.enter_context(tc.tile_pool(name="bstage", bufs=6))
        xch = ctx.enter_context(tc.tile_pool(name="xch", bufs=3))
        persist = ctx.enter_context(tc.tile_pool(name="persist", bufs=1))
        apool = ctx.enter_context(tc.tile_pool(name="apool", bufs=40))
        small = ctx.enter_context(tc.tile_pool(name="small", bufs=2))
        bcast = ctx.enter_context(tc.tile_pool(name="bcast", bufs=4))
        work = ctx.enter_context(tc.tile_pool(name="work", bufs=2))
        oscp = ctx.enter_context(tc.tile_pool(name="oscp", bufs=3))
        yout = ctx.enter_context(tc.tile_pool(name="yout", bufs=2))
        ps_a = ctx.enter_context(tc.tile_pool(name="ps_a", bufs=2, space="PSUM"))
        ps_o = ctx.enter_context(tc.tile_pool(name="ps_o", bufs=3, space="PSUM"))
        ps_s = ctx.enter_context(tc.tile_pool(name="ps_s", bufs=1, space="PSUM"))
        ps_y = ctx.enter_context(tc.tile_pool(name="ps_y", bufs=1, space="PSUM"))

        # ---------------- constants ----------------
        mask0 = const.tile([128, 128], BF16, tag="mask0")
        make_upper_triangular(nc, mask0[:], val=1.0, diag=True)
        ones_bf = const.tile([128, 1], BF16, tag="ones")
        nc.vector.memset(ones_bf[:], 1.0)
        ones_row = const.tile([1, 128], F32, tag="ones_row")
        nc.vector.memset(ones_row[:], 1.0)

        lamt = {}
        for nm in ("lq1", "lq2", "lk1", "lk2"):
            t_ = const.tile([1, HHD], F32, tag=nm)
            nc.sync.dma_start(t_[:], lam_d[nm])
            lamt[nm] = t_
        evals = []
        for a, b in (("lq1", "lk1"), ("lq2", "lk2")):
            m_ = small.tile([1, HHD], F32, tag="lmul")
            nc.vector.tensor_mul(m_[:], lamt[a][:], lamt[b][:])
            s_ = small.tile([1, 1], F32, tag="lsum")
            nc.vector.tensor_reduce(s_[:], m_[:], axis=mybir.AxisListType.X,
                                    op=mybir.AluOpType.add)
            e_ = const.tile([1, 1], F32, tag=f"e_{a}")
            nc.scalar.activation(e_[:], s_[:], AF.Exp)
            evals.append(e_)
        # neglam = -(exp(s1) - exp(s2) + LAMBDA_INIT) = e2 - e1 - LAMBDA_INIT
        neglam = const.tile([1, 1], F32, tag="neglam")
        nc.vector.tensor_sub(neglam[:], evals[1][:], evals[0][:])
        nc.vector.tensor_scalar_add(neglam[:], neglam[:], -LAMBDA_INIT)

        eps_t = const.tile([1, 1], F32, tag="eps")
        nc.vector.memset(eps_t[:], 1e-5)
        rms_t = const.tile([128, 1], F32, tag="rms")
        nc.sync.dma_start(rms_t[:], rms_d)
        woscale = const.tile([128, 1], F32, tag="wos")
        nc.vector.tensor_scalar_mul(woscale[:], rms_t[:], 1.0 - LAMBDA_INIT)

        # ---------------- weights ----------------
        wq_bf = wpool.tile([128, 8, 256], BF16, tag="wq")
        wk_bf = wpool.tile([128, 8, 256], BF16, tag="wk")
        wv_bf = wpool.tile([128, 8, 256], BF16, tag="wv")
        for wd, wt in ((wq_d, wq_bf), (wk_d, wk_bf), (wv_d, wv_bf)):
            for g in range(8):
                ws = stage.tile([128, 256], F32, tag="wstg")
                nc.sync.dma_start(ws[:], wd[128 * g:128 * (g + 1), :])
                nc.vector.tensor_copy(wt[:, g, :], ws[:])
        wo_bf = wpool.tile([128, 2, C], BF16, tag="wo")
        for h in range(2):
            ws = stage.tile([128, C], F32, tag="stg")
            nc.sync.dma_start(ws[:], wo_d[128 * h:128 * (h + 1), :])
            nc.vector.tensor_scalar_mul(ws[:], ws[:], woscale[:])
            nc.vector.tensor_copy(wo_bf[:, h, :], ws[:])

        # ---- fused per-chunk pipeline: loads/projections j overlap attention ----
        V_all = persist.tile([128, NT, 256], BF16, tag="V_all")
        kT_h = [persist.tile([128, T], BF16, tag=f"kT{h}", name=f"kT{h}") for h in range(2)]
        qT_h = [persist.tile([128, T], BF16, tag=f"qT{h}", name=f"qT{h}") for h in range(2)]
        for j in range(NCH):
            # value rows 512j..512(j+1) -> V_all n-tiles 4j..4j+3
            for ii in range(4):
                i = 4 * j + ii
                bst = bstage.tile([128, C], BF16, tag="bstg")
                nc.gpsimd.dma_start(bst[:], v_d[128 * i:128 * (i + 1), :])
                xv = xch.tile([128, 8, 128], BF16, tag="xsm")
                nc.sync.dma_start(xv[:], bst[:], transpose=True)
                pv = ps_a.tile([128, 512], F32, tag="ps_big")
                for g in range(8):
                    nc.tensor.matmul(pv[:, :256], xv[:, g, :], wv_bf[:, g, :],
                                     start=(g == 0), stop=(g == 7))
                nc.vector.tensor_copy(V_all[:, i, :], pv[:, :256])
            # key chunk j -> kT_h[:, 512j:512(j+1)]
            xk = xch.tile([128, 8, TCH], BF16, tag="xbig")
            for kk in range(4):
                gt = 4 * j + kk
                bst = bstage.tile([128, C], BF16, tag="bstg")
                nc.gpsimd.dma_start(bst[:], k_d[128 * gt:128 * (gt + 1), :])
                nc.sync.dma_start(xk[:, :, 128 * kk:128 * (kk + 1)], bst[:],
                                  transpose=True)
            for ds in range(2):
                pk = ps_a.tile([128, 512], F32, tag="ps_big")
                for g in range(8):
                    nc.tensor.matmul(pk[:], wk_bf[:, g, 128 * ds:128 * (ds + 1)],
                                     xk[:, g, :], start=(g == 0), stop=(g == 7))
                nc.scalar.copy(kT_h[ds][:, TCH * j:TCH * (j + 1)], pk[:])
            # query chunk j -> qT_h[:, 512j:512(j+1)]
            xq = xch.tile([128, 8, TCH], BF16, tag="xbig")
            for kk in range(4):
                gt = 4 * j + kk
                bst = bstage.tile([128, C], BF16, tag="bstg")
                nc.gpsimd.dma_start(bst[:], q_d[128 * gt:128 * (gt + 1), :])
                nc.sync.dma_start(xq[:, :, 128 * kk:128 * (kk + 1)], bst[:],
                                  transpose=True)
            for ds in range(2):
                pq = ps_a.tile([128, 512], F32, tag="ps_big")
                for g in range(8):
                    nc.tensor.matmul(pq[:], wq_bf[:, g, 128 * ds:128 * (ds + 1)],
                                     xq[:, g, :], start=(g == 0), stop=(g == 7))
                nc.scalar.copy(qT_h[ds][:, TCH * j:TCH * (j + 1)], pq[:])

            n_hi = 4 * j + 4
            # ---- phase A: per-head matmul blocks (scores/exp/O/rowsum) ----
            o_pss = []
            r_pss = []
            for h in range(2):
                A = {}
                for i in range(n_hi):
                    ts0 = max(0, 128 * i - TCH * j)
                    for hh in range(2):
                        r0 = 64 * hh
                        sp = ps_a.tile([128, 512], F32, tag="ps_big")
                        nc.tensor.matmul(
                            sp[:, ts0:], kT_h[h][r0:r0 + 64, 128 * i:128 * (i + 1)],
                            qT_h[h][r0:r0 + 64, TCH * j + ts0:TCH * (j + 1)])
                        at = apool.tile([128, 512], BF16, tag="at")
                        nc.scalar.activation(at[:, ts0:], sp[:, ts0:], AF.Exp,
                                             scale=SCALE)
                        if i >= 4 * j:
                            nc.vector.tensor_mul(at[:, ts0:ts0 + 128],
                                                 at[:, ts0:ts0 + 128], mask0[:])
                        A[(hh, i)] = (at, ts0)
                o_ps = [ps_o.tile([128, 512], F32, tag="ops", name=f"ops{_h}")
                        for _h in range(2)]
                for i in range(n_hi):
                    for hh in range(2):
                        at, ts0 = A[(hh, i)]
                        nc.tensor.matmul(o_ps[hh][:, ts0:],
                                         V_all[:, i, 128 * h:128 * (h + 1)],
                                         at[:, ts0:], start=(i == 0),
                                         stop=(i == n_hi - 1))
                # rowsums: rows 0 / 32 = hh0 / hh1 of one psum bank
                r_ps = ps_s.tile([33, 512], F32, tag="rps")
                for i in range(n_hi):
                    for hh in range(2):
                        at, ts0 = A[(hh, i)]
                        ro = 32 * hh
                        nc.tensor.matmul(r_ps[ro:ro + 1, ts0:], ones_bf[:],
                                         at[:, ts0:], start=(i == 0),
                                         stop=(i == n_hi - 1))
                o_pss.append(o_ps)
                r_pss.append(r_ps)
            # ---- phase B: per-head softmax-division/combine/rms tails ----
            osc = []
            ocs = []
            ms_list = []
            for h in range(2):
                o_ps, r_ps = o_pss[h], r_pss[h]
                rv1 = small.tile([1, 512], F32, tag="rv1")
                nc.vector.reciprocal(rv1[:], r_ps[0:1, :])
                rv2 = small.tile([1, 512], F32, tag="rv2")
                nc.vector.reciprocal(rv2[:], r_ps[32:33, :])
                b2p = small.tile([1, 512], F32, tag="b2p")
                nc.vector.tensor_scalar_mul(b2p[:], rv2[:], neglam[:])
                bp1 = ps_y.tile([128, 512], F32, tag="ypb")
                nc.tensor.matmul(bp1[:], ones_row[:], rv1[:])
                B1 = bcast.tile([128, 512], F32, tag="B")
                nc.vector.tensor_copy(B1[:], bp1[:])
                bp2 = ps_y.tile([128, 512], F32, tag="ypb")
                nc.tensor.matmul(bp2[:], ones_row[:], b2p[:])
                B2 = bcast.tile([128, 512], F32, tag="B")
                nc.vector.tensor_copy(B2[:], bp2[:])
                t1 = work.tile([128, 512], F32, tag="wk1")
                nc.vector.tensor_mul(t1[:], o_ps[0][:], B1[:])
                t2 = work.tile([128, 512], F32, tag="wk2")
                nc.vector.tensor_mul(t2[:], o_ps[1][:], B2[:])
                oc = work.tile([128, 512], F32, tag="oc")
                nc.vector.tensor_add(oc[:], t1[:], t2[:])
                osq = work.tile([128, 512], BF16, tag="osq")
                nc.vector.tensor_mul(osq[:], oc[:], oc[:])
                ms = ps_s.tile([1, 512], F32, tag="msps")
                nc.tensor.matmul(ms[:], ones_bf[:], osq[:])
                ocs.append(oc)
                ms_list.append(ms)
            for h in range(2):
                sq = small.tile([1, 512], F32, tag="sq")
                nc.scalar.activation(sq[:], ms_list[h][:], AF.Sqrt,
                                     bias=eps_t[:], scale=1.0 / DH)
                sv = small.tile([1, 512], F32, tag="sv")
                nc.vector.reciprocal(sv[:], sq[:])
                bp3 = ps_y.tile([128, 512], F32, tag="ypb")
                nc.tensor.matmul(bp3[:], ones_row[:], sv[:])
                SB = bcast.tile([128, 512], F32, tag="B")
                nc.vector.tensor_copy(SB[:], bp3[:])
                os_t = oscp.tile([128, 512], BF16, tag="osc")
                nc.vector.tensor_mul(os_t[:], ocs[h][:], SB[:])
                osc.append(os_t)
            # output projection: Y[t, :] = sum_h O_h^T(t)^T @ Wo_h
            for k4 in range(4):
                ysb = yout.tile([128, C], F32, tag="ysb")
                for half in range(2):
                    yp = ps_y.tile([128, 512], F32, tag="ypb")
                    for h in range(2):
                        lt = osc[h][:, 128 * k4:128 * (k4 + 1)]
                        nc.tensor.matmul(yp[:], lt,
                                         wo_bf[:, h, 512 * half:512 * (half + 1)],
                                         start=(h == 0), stop=(h == 1))
                    nc.vector.tensor_copy(ysb[:, 512 * half:512 * (half + 1)],
                                          yp[:])
                row = TCH * j + 128 * k4
                nc.sync.dma_start(y_d[row:row + 128, :], ysb[:])

    nc.compile()
    return nc


_NC_CACHE = None


def get_nc():
    global _NC_CACHE
    if _NC_CACHE is None:
        _NC_CACHE = build_nc()
    return _NC_CACHE


def make_in_maps(query, key_t, value, Wq, Wk, Wv, Wo,
                 lambda_q1, lambda_q2, lambda_k1, lambda_k2, rms_weight):
    f = np.float32
    in_maps = []
    for core in range(8):
        b, p = divmod(core, 4)
        sl = slice(256 * p, 256 * (p + 1))
        in_maps.append({
            "query": np.ascontiguousarray(query[b], dtype=f),
            "key": np.ascontiguousarray(key_t[b], dtype=f),
            "value": np.ascontiguousarray(value[b], dtype=f),
            "wq": np.ascontiguousarray(Wq[:, sl], dtype=f),
            "wk": np.ascontiguousarray(Wk[:, sl], dtype=f),
            "wv": np.ascontiguousarray(Wv[:, sl], dtype=f),
            "wo": np.ascontiguousarray(Wo[sl, :], dtype=f),
            "lq1": np.ascontiguousarray(lambda_q1, dtype=f).reshape(1, HHD),
            "lq2": np.ascontiguousarray(lambda_q2, dtype=f).reshape(1, HHD),
            "lk1": np.ascontiguousarray(lambda_k1, dtype=f).reshape(1, HHD),
            "lk2": np.ascontiguousarray(lambda_k2, dtype=f).reshape(1, HHD),
            "rmsw": np.ascontiguousarray(rms_weight, dtype=f).reshape(128, 1),
        })
    return in_maps


def kernel(query, key_t, value, Wq, Wk, Wv, Wo,
           lambda_q1, lambda_q2, lambda_k1, lambda_k2, rms_weight):
    in_maps = make_in_maps(query, key_t, value, Wq, Wk, Wv, Wo,
                           lambda_q1, lambda_q2, lambda_k1, lambda_k2,
                           rms_weight)
    res = run_bass_kernel_spmd(get_nc(), in_maps, core_ids=list(range(8)))
    parts = np.stack([res.results[i]["y"] for i in range(8)])
    return parts.reshape(2, 4, T, C).sum(axis=1).astype(np.float32)


# revision 12
# speedup vs baseline: 1.8678x; 1.8678x over previous
"""Differential multi-head attention (DiffMHA) Trainium2 kernel.

Full-input contract: kernel(**inputs) takes the unsharded numpy inputs and
returns the full [2, 2048, 1024] f32 output. Internally the work is sharded
across 8 NeuronCores: data-parallel over the batch (B=2) and tensor-parallel
over heads (2 value heads / 4 score half-heads per core), with Wq/Wk/Wv
column-sharded and Wo row-sharded. Each core produces a full-width partial
Y contribution; the host sums the 4 head-group partials per batch element
(the "all-reduce" of Megatron row-parallelism, done on host for free).

Per-core pipeline (all matmuls bf16 x bf16 -> f32 PSUM):
  1. inputs f32 -> SBUF, cast bf16, DMA-transpose to [c, t] layout
  2. qT/kT = Wq^T X^T ([d, t]), V = X Wv (natural [n, e])
  3. per 512-wide t-chunk: scores^T = kT^T qT per half-head (causal-skipped),
     exp on ScalarE, rowsums via ones-matmul, O^T = V^T A^T accumulation,
     deferred softmax division + differential-lambda combine on VectorE,
     RMS norm via ones-matmul of squares + partition-broadcast scale
  4. Y += O_scaled^T^T @ Wo_scaled  (rms_weight * (1-lambda_init) is folded
     into Wo rows on-chip), DMA out.
"""

import math
from contextlib import ExitStack

import numpy as np

import concourse.bacc as bacc
import concourse.mybir as mybir
import concourse.tile as tile
from concourse.bass_utils import run_bass_kernel_spmd
from concourse.masks import make_upper_triangular

F32 = mybir.dt.float32
BF16 = mybir.dt.bfloat16
AF = mybir.ActivationFunctionType

T = 2048
C = 1024
DH = 128          # value-head dim
HHD = 64          # score half-head dim
NCH = 4           # t-chunks of 512
TCH = 512
NT = 16           # n-tiles of 128
LAMBDA_INIT = 0.8 - 0.6 * math.exp(-0.3 * 12)
SCALE = 1.0 / math.sqrt(HHD)


def build_nc():
    nc = bacc.Bacc("TRN2", target_bir_lowering=False, debug=False,
                   enable_asserts=False)
    q_d = nc.dram_tensor("query", [T, C], F32, kind="ExternalInput").ap()
    k_d = nc.dram_tensor("key", [T, C], F32, kind="ExternalInput").ap()
    v_d = nc.dram_tensor("value", [T, C], F32, kind="ExternalInput").ap()
    wq_d = nc.dram_tensor("wq", [C, 256], F32, kind="ExternalInput").ap()
    wk_d = nc.dram_tensor("wk", [C, 256], F32, kind="ExternalInput").ap()
    wv_d = nc.dram_tensor("wv", [C, 256], F32, kind="ExternalInput").ap()
    wo_d = nc.dram_tensor("wo", [256, C], F32, kind="ExternalInput").ap()
    lam_d = {nm: nc.dram_tensor(nm, [1, HHD], F32, kind="ExternalInput").ap()
             for nm in ("lq1", "lq2", "lk1", "lk2")}
    rms_d = nc.dram_tensor("rmsw", [128, 1], F32, kind="ExternalInput").ap()
    y_d = nc.dram_tensor("y", [T, C], F32, kind="ExternalOutput").ap()

    with tile.TileContext(nc) as tc, ExitStack() as ctx:
        const = ctx.enter_context(tc.tile_pool(name="const", bufs=1))
        wpool = ctx.enter_context(tc.tile_pool(name="wpool", bufs=1))
        stage = ctx.enter_context(tc.tile_pool(name="stage", bufs=3))
        bstage = ctx.enter_context(tc.tile_pool(name="bstage", bufs=6))
        xch = ctx.enter_context(tc.tile_pool(name="xch", bufs=3))
        persist = ctx.enter_context(tc.tile_pool(name="persist", bufs=1))
        apool = ctx.enter_context(tc.tile_pool(name="apool", bufs=40))
        small = ctx.enter_context(tc.tile_pool(name="small", bufs=2))
        bcast = ctx.enter_context(tc.tile_pool(name="bcast", bufs=4))
        work = ctx.enter_context(tc.tile_pool(name="work", bufs=2))
        oscp = ctx.enter_context(tc.tile_pool(name="oscp", bufs=3))
        yout = ctx.enter_context(tc.tile_pool(name="yout", bufs=2))
        ps_a = ctx.enter_context(tc.tile_pool(name="ps_a", bufs=2, space="PSUM"))
        ps_p = ctx.enter_context(tc.tile_pool(name="ps_p", bufs=1, space="PSUM"))
        ps_o = ctx.enter_context(tc.tile_pool(name="ps_o", bufs=2, space="PSUM"))
        ps_s = ctx.enter_context(tc.tile_pool(name="ps_s", bufs=2, space="PSUM"))
        ps_y = ctx.enter_context(tc.tile_pool(name="ps_y", bufs=1, space="PSUM"))

        # ---------------- constants ----------------
        mask0 = const.tile([128, 128], BF16, tag="mask0")
        make_upper_triangular(nc, mask0[:], val=1.0, diag=True)
        ones_bf = const.tile([128, 1], BF16, tag="ones")
        nc.vector.memset(ones_bf[:], 1.0)
        ones_row = const.tile([1, 128], F32, tag="ones_row")
        nc.vector.memset(ones_row[:], 1.0)

        lamt = {}
        for nm in ("lq1", "lq2", "lk1", "lk2"):
            t_ = const.tile([1, HHD], F32, tag=nm)
            nc.sync.dma_start(t_[:], lam_d[nm])
            lamt[nm] = t_
        evals = []
        for a, b in (("lq1", "lk1"), ("lq2", "lk2")):
            m_ = small.tile([1, HHD], F32, tag="lmul")
            nc.vector.tensor_mul(m_[:], lamt[a][:], lamt[b][:])
            s_ = small.tile([1, 1], F32, tag="lsum")
            nc.vector.tensor_reduce(s_[:], m_[:], axis=mybir.AxisListType.X,
                                    op=mybir.AluOpType.add)
            e_ = const.tile([1, 1], F32, tag=f"e_{a}")
            nc.scalar.activation(e_[:], s_[:], AF.Exp)
            evals.append(e_)
        # neglam = -(exp(s1) - exp(s2) + LAMBDA_INIT) = e2 - e1 - LAMBDA_INIT
        neglam = const.tile([1, 1], F32, tag="neglam")
        nc.vector.tensor_sub(neglam[:], evals[1][:], evals[0][:])
        nc.vector.tensor_scalar_add(neglam[:], neglam[:], -LAMBDA_INIT)

        eps_t = const.tile([1, 1], F32, tag="eps")
        nc.vector.memset(eps_t[:], 1e-5)
        rms_t = const.tile([128, 1], F32, tag="rms")
        nc.sync.dma_start(rms_t[:], rms_d)
        woscale = const.tile([128, 1], F32, tag="wos")
        nc.vector.tensor_scalar_mul(woscale[:], rms_t[:], 1.0 - LAMBDA_INIT)

        # ---------------- weights ----------------
        wq_bf = wpool.tile([128, 8, 256], BF16, tag="wq")
        wk_bf = wpool.tile([128, 8, 256], BF16, tag="wk")
        wv_bf = wpool.tile([128, 8, 256], BF16, tag="wv")
        for wd, wt in ((wq_d, wq_bf), (wk_d, wk_bf), (wv_d, wv_bf)):
            for g in range(8):
                ws = stage.tile([128, 256], F32, tag="wstg")
                nc.sync.dma_start(ws[:], wd[128 * g:128 * (g + 1), :])
                nc.vector.tensor_copy(wt[:, g, :], ws[:])
        wo_bf = wpool.tile([128, 2, C], BF16, tag="wo")
        for h in range(2):
            ws = stage.tile([128, C], F32, tag="stg")
            nc.sync.dma_start(ws[:], wo_d[128 * h:128 * (h + 1), :])
            nc.vector.tensor_scalar_mul(ws[:], ws[:], woscale[:])
            nc.vector.tensor_copy(wo_bf[:, h, :], ws[:])

        # ---- fused per-chunk pipeline: loads/projections j overlap attention ----
        V_all = persist.tile([128, NT, 256], BF16, tag="V_all")
        kT_h = [persist.tile([128, T], BF16, tag=f"kT{h}", name=f"kT{h}") for h in range(2)]
        qT_h = [persist.tile([128, T], BF16, tag=f"qT{h}", name=f"qT{h}") for h in range(2)]
        for j in range(NCH):
            # value rows 512j..512(j+1) -> V_all n-tiles 4j..4j+3
            for ii in range(4):
                i = 4 * j + ii
                bst = bstage.tile([128, C], BF16, tag="bstg")
                nc.gpsimd.dma_start(bst[:], v_d[128 * i:128 * (i + 1), :])
                xv = xch.tile([128, 8, 128], BF16, tag="xsm")
                nc.sync.dma_start(xv[:], bst[:], transpose=True)
                pv = ps_p.tile([128, 512], F32, tag="pp")
                for g in range(8):
                    nc.tensor.matmul(pv[:, :256], xv[:, g, :], wv_bf[:, g, :],
                                     start=(g == 0), stop=(g == 7))
                nc.vector.tensor_copy(V_all[:, i, :], pv[:, :256])
            # key chunk j -> kT_h[:, 512j:512(j+1)]
            xk = xch.tile([128, 8, TCH], BF16, tag="xbig")
            for kk in range(4):
                gt = 4 * j + kk
                bst = bstage.tile([128, C], BF16, tag="bstg")
                nc.gpsimd.dma_start(bst[:], k_d[128 * gt:128 * (gt + 1), :])
                nc.sync.dma_start(xk[:, :, 128 * kk:128 * (kk + 1)], bst[:],
                                  transpose=True)
            for ds in range(2):
                pk = ps_p.tile([128, 512], F32, tag="pp")
                for g in range(8):
                    nc.tensor.matmul(pk[:], wk_bf[:, g, 128 * ds:128 * (ds + 1)],
                                     xk[:, g, :], start=(g == 0), stop=(g == 7))
                nc.scalar.copy(kT_h[ds][:, TCH * j:TCH * (j + 1)], pk[:])
            # query chunk j -> qT_h[:, 512j:512(j+1)]
            xq = xch.tile([128, 8, TCH], BF16, tag="xbig")
            for kk in range(4):
                gt = 4 * j + kk
                bst = bstage.tile([128, C], BF16, tag="bstg")
                nc.gpsimd.dma_start(bst[:], q_d[128 * gt:128 * (gt + 1), :])
                nc.sync.dma_start(xq[:, :, 128 * kk:128 * (kk + 1)], bst[:],
                                  transpose=True)
            for ds in range(2):
                pq = ps_p.tile([128, 512], F32, tag="pp")
                for g in range(8):
                    nc.tensor.matmul(pq[:], wq_bf[:, g, 128 * ds:128 * (ds + 1)],
                                     xq[:, g, :], start=(g == 0), stop=(g == 7))
                nc.scalar.copy(qT_h[ds][:, TCH * j:TCH * (j + 1)], pq[:])

            n_hi = 4 * j + 4
            # ---- phase A: per-head matmul blocks (scores/exp/O/rowsum) ----
            o_pss = []
            r_pss = []
            for h in range(2):
                A = {}
                for i in range(n_hi):
                    ts0 = max(0, 128 * i - TCH * j)
                    for hh in range(2):
                        r0 = 64 * hh
                        sp = ps_a.tile([128, 512], F32, tag="sps")
                        nc.tensor.matmul(
                            sp[:, ts0:], kT_h[h][r0:r0 + 64, 128 * i:128 * (i + 1)],
                            qT_h[h][r0:r0 + 64, TCH * j + ts0:TCH * (j + 1)])
                        at = apool.tile([128, 512], BF16, tag="at")
                        nc.scalar.activation(at[:, ts0:], sp[:, ts0:], AF.Exp,
                                             scale=SCALE)
                        if i >= 4 * j:
                            nc.vector.tensor_mul(at[:, ts0:ts0 + 128],
                                                 at[:, ts0:ts0 + 128], mask0[:])
                        A[(hh, i)] = (at, ts0)
                o_ps = [ps_o.tile([128, 512], F32, tag="ops", name=f"ops{_h}")
                        for _h in range(2)]
                for i in range(n_hi):
                    for hh in range(2):
                        at, ts0 = A[(hh, i)]
                        nc.tensor.matmul(o_ps[hh][:, ts0:],
                                         V_all[:, i, 128 * h:128 * (h + 1)],
                                         at[:, ts0:], start=(i == 0),
                                         stop=(i == n_hi - 1))
                # rowsums rows 0/32 = hh0/hh1; row 64 = rms square-sum
                r_ps = ps_s.tile([65, 512], F32, tag="rps")
                for i in range(n_hi):
                    for hh in range(2):
                        at, ts0 = A[(hh, i)]
                        ro = 32 * hh
                        nc.tensor.matmul(r_ps[ro:ro + 1, ts0:], ones_bf[:],
                                         at[:, ts0:], start=(i == 0),
                                         stop=(i == n_hi - 1))
                o_pss.append(o_ps)
                r_pss.append(r_ps)
            # ---- phase B: per-head softmax-division/combine/rms tails ----
            osc = []
            ocs = []
            for h in range(2):
                o_ps, r_ps = o_pss[h], r_pss[h]
                rv1 = small.tile([1, 512], F32, tag="rv1")
                nc.vector.reciprocal(rv1[:], r_ps[0:1, :])
                rv2 = small.tile([1, 512], F32, tag="rv2")
                nc.vector.reciprocal(rv2[:], r_ps[32:33, :])
                b2p = small.tile([1, 512], F32, tag="b2p")
                nc.vector.tensor_scalar_mul(b2p[:], rv2[:], neglam[:])
                bp1 = ps_y.tile([128, 512], F32, tag="ypb")
                nc.tensor.matmul(bp1[:], ones_row[:], rv1[:])
                B1 = bcast.tile([128, 512], F32, tag="B")
                nc.vector.tensor_copy(B1[:], bp1[:])
                bp2 = ps_y.tile([128, 512], F32, tag="ypb")
                nc.tensor.matmul(bp2[:], ones_row[:], b2p[:])
                B2 = bcast.tile([128, 512], F32, tag="B")
                nc.vector.tensor_copy(B2[:], bp2[:])
                t1 = work.tile([128, 512], F32, tag="wk1")
                nc.vector.tensor_mul(t1[:], o_ps[0][:], B1[:])
                t2 = work.tile([128, 512], F32, tag="wk2")
                nc.vector.tensor_mul(t2[:], o_ps[1][:], B2[:])
                oc = work.tile([128, 512], F32, tag="oc")
                nc.vector.tensor_add(oc[:], t1[:], t2[:])
                osq = work.tile([128, 512], BF16, tag="osq")
                nc.vector.tensor_mul(osq[:], oc[:], oc[:])
                nc.tensor.matmul(r_ps[64:65, :], ones_bf[:], osq[:])
                ocs.append(oc)
            for h in range(2):
                sq = small.tile([1, 512], F32, tag="sq")
                nc.scalar.activation(sq[:], r_pss[h][64:65, :], AF.Sqrt,
                                     bias=eps_t[:], scale=1.0 / DH)
                sv = small.tile([1, 512], F32, tag="sv")
                nc.vector.reciprocal(sv[:], sq[:])
                bp3 = ps_y.tile([128, 512], F32, tag="ypb")
                nc.tensor.matmul(bp3[:], ones_row[:], sv[:])
                SB = bcast.tile([128, 512], F32, tag="B")
                nc.vector.tensor_copy(SB[:], bp3[:])
                os_t = oscp.tile([128, 512], BF16, tag="osc")
                nc.vector.tensor_mul(os_t[:], ocs[h][:], SB[:])
                osc.append(os_t)
            # output projection: Y[t, :] = sum_h O_h^T(t)^T @ Wo_h
            for k4 in range(4):
                ysb = yout.tile([128, C], F32, tag="ysb")
                for half in range(2):
                    yp = ps_y.tile([128, 512], F32, tag="ypb")
                    for h in range(2):
                        lt = osc[h][:, 128 * k4:128 * (k4 + 1)]
                        nc.tensor.matmul(yp[:], lt,
                                         wo_bf[:, h, 512 * half:512 * (half + 1)],
                                         start=(h == 0), stop=(h == 1))
                    nc.vector.tensor_copy(ysb[:, 512 * half:512 * (half + 1)],
                                          yp[:])
                row = TCH * j + 128 * k4
                nc.sync.dma_start(y_d[row:row + 128, :], ysb[:])

    nc.compile()
    return nc


_NC_CACHE = None


def get_nc():
    global _NC_CACHE
    if _NC_CACHE is None:
        _NC_CACHE = build_nc()
    return _NC_CACHE


def make_in_maps(query, key_t, value, Wq, Wk, Wv, Wo,
                 lambda_q1, lambda_q2, lambda_k1, lambda_k2, rms_weight):
    f = np.float32
    in_maps = []
    for core in range(8):
        b, p = divmod(core, 4)
        sl = slice(256 * p, 256 * (p + 1))
        in_maps.append({
            "query": np.ascontiguousarray(query[b], dtype=f),
            "key": np.ascontiguousarray(key_t[b], dtype=f),
            "value": np.ascontiguousarray(value[b], dtype=f),
            "wq": np.ascontiguousarray(Wq[:, sl], dtype=f),
            "wk": np.ascontiguousarray(Wk[:, sl], dtype=f),
            "wv": np.ascontiguousarray(Wv[:, sl], dtype=f),
            "wo": np.ascontiguousarray(Wo[sl, :], dtype=f),
            "lq1": np.ascontiguousarray(lambda_q1, dtype=f).reshape(1, HHD),
            "lq2": np.ascontiguousarray(lambda_q2, dtype=f).reshape(1, HHD),
            "lk1": np.ascontiguousarray(lambda_k1, dtype=f).reshape(1, HHD),
            "lk2": np.ascontiguousarray(lambda_k2, dtype=f).reshape(1, HHD),
            "rmsw": np.ascontiguousarray(rms_weight, dtype=f).reshape(128, 1),
        })
    return in_maps


def kernel(query, key_t, value, Wq, Wk, Wv, Wo,
           lambda_q1, lambda_q2, lambda_k1, lambda_k2, rms_weight):
    in_maps = make_in_maps(query, key_t, value, Wq, Wk, Wv, Wo,
                           lambda_q1, lambda_q2, lambda_k1, lambda_k2,
                           rms_weight)
    res = run_bass_kernel_spmd(get_nc(), in_maps, core_ids=list(range(8)))
    parts = np.stack([res.results[i]["y"] for i in range(8)])
    return parts.reshape(2, 4, T, C).sum(axis=1).astype(np.float32)


def build_nc_baseline():
    """Same I/O signature, near-empty body — for dispatch-overhead timing."""
    nc = bacc.Bacc("TRN2", target_bir_lowering=False, debug=False,
                   enable_asserts=False)
    nc.dram_tensor("query", [T, C], F32, kind="ExternalInput")
    nc.dram_tensor("key", [T, C], F32, kind="ExternalInput")
    nc.dram_tensor("value", [T, C], F32, kind="ExternalInput")
    nc.dram_tensor("wq", [C, 256], F32, kind="ExternalInput")
    nc.dram_tensor("wk", [C, 256], F32, kind="ExternalInput")
    nc.dram_tensor("wv", [C, 256], F32, kind="ExternalInput")
    nc.dram_tensor("wo", [256, C], F32, kind="ExternalInput")
    for nm in ("lq1", "lq2", "lk1", "lk2"):
        nc.dram_tensor(nm, [1, HHD], F32, kind="ExternalInput")
    nc.dram_tensor("rmsw", [128, 1], F32, kind="ExternalInput")
    y_d = nc.dram_tensor("y", [T, C], F32, kind="ExternalOutput").ap()
    with tile.TileContext(nc) as tc, ExitStack() as ctx:
        pool = ctx.enter_context(tc.tile_pool(name="p", bufs=1))
        t_ = pool.tile([128, C], F32, tag="t")
        nc.vector.memset(t_[:], 0.0)
        nc.sync.dma_start(y_d[0:128, :], t_[:])
    nc.compile()
    return nc


# revision 13
# speedup vs baseline: 12.3138x; 6.5925x over previous
"""Differential multi-head attention (DiffMHA) Trainium2 kernel.

Full-input contract: kernel(**inputs) takes the unsharded numpy inputs and
returns the full [2, 2048, 1024] f32 output. Internally the work is sharded
across 8 NeuronCores: data-parallel over the batch (B=2) and tensor-parallel
over heads (2 value heads / 4 score half-heads per core), with Wq/Wk/Wv
column-sharded and Wo row-sharded. Each core produces a full-width partial
Y contribution; the host sums the 4 head-group partials per batch element
(the "all-reduce" of Megatron row-parallelism, done on host for free).

Per-core pipeline (all matmuls bf16 x bf16 -> f32 PSUM):
  1. inputs f32 -> SBUF, cast bf16, DMA-transpose to [c, t] layout
  2. qT/kT = Wq^T X^T ([d, t]), V = X Wv (natural [n, e])
  3. per 512-wide t-chunk: scores^T = kT^T qT per half-head (causal-skipped),
     exp on ScalarE, rowsums via ones-matmul, O^T = V^T A^T accumulation,
     deferred softmax division + differential-lambda combine on VectorE,
     RMS norm via ones-matmul of squares + partition-broadcast scale
  4. Y += O_scaled^T^T @ Wo_scaled  (rms_weight * (1-lambda_init) is folded
     into Wo rows on-chip), DMA out.
"""

import math
from contextlib import ExitStack

import numpy as np

import concourse.bacc as bacc
import concourse.mybir as mybir
import concourse.tile as tile
from concourse.bass_utils import run_bass_kernel_spmd
from concourse.masks import make_upper_triangular

F32 = mybir.dt.float32
BF16 = mybir.dt.bfloat16
AF = mybir.ActivationFunctionType

T = 2048
C = 1024
DH = 128          # value-head dim
HHD = 64          # score half-head dim
NCH = 4           # t-chunks of 512
TCH = 512
NT = 16           # n-tiles of 128
LAMBDA_INIT = 0.8 - 0.6 * math.exp(-0.3 * 12)
SCALE = 1.0 / math.sqrt(HHD)


def build_nc(repeats=1):
    nc = bacc.Bacc("TRN2", target_bir_lowering=False, debug=False,
                   enable_asserts=False)
    q_d = nc.dram_tensor("query", [T, C], F32, kind="ExternalInput").ap()
    k_d = nc.dram_tensor("key", [T, C], F32, kind="ExternalInput").ap()
    v_d = nc.dram_tensor("value", [T, C], F32, kind="ExternalInput").ap()
    wq_d = nc.dram_tensor("wq", [C, 256], F32, kind="ExternalInput").ap()
    wk_d = nc.dram_tensor("wk", [C, 256], F32, kind="ExternalInput").ap()
    wv_d = nc.dram_tensor("wv", [C, 256], F32, kind="ExternalInput").ap()
    wo_d = nc.dram_tensor("wo", [256, C], F32, kind="ExternalInput").ap()
    lam_d = {nm: nc.dram_tensor(nm, [1, HHD], F32, kind="ExternalInput").ap()
             for nm in ("lq1", "lq2", "lk1", "lk2")}
    rms_d = nc.dram_tensor("rmsw", [128, 1], F32, kind="ExternalInput").ap()
    y_d = nc.dram_tensor("y", [T, C], F32, kind="ExternalOutput").ap()

    with tile.TileContext(nc) as tc, ExitStack() as ctx:
        const = ctx.enter_context(tc.tile_pool(name="const", bufs=1))
        wpool = ctx.enter_context(tc.tile_pool(name="wpool", bufs=1))
        stage = ctx.enter_context(tc.tile_pool(name="stage", bufs=3))
        bstage = ctx.enter_context(tc.tile_pool(name="bstage", bufs=6))
        xch = ctx.enter_context(tc.tile_pool(name="xch", bufs=3))
        persist = ctx.enter_context(tc.tile_pool(name="persist", bufs=1))
        apool = ctx.enter_context(tc.tile_pool(name="apool", bufs=40))
        small = ctx.enter_context(tc.tile_pool(name="small", bufs=2))
        bcast = ctx.enter_context(tc.tile_pool(name="bcast", bufs=4))
        work = ctx.enter_context(tc.tile_pool(name="work", bufs=2))
        oscp = ctx.enter_context(tc.tile_pool(name="oscp", bufs=3))
        yout = ctx.enter_context(tc.tile_pool(name="yout", bufs=2))
        ps_a = ctx.enter_context(tc.tile_pool(name="ps_a", bufs=2, space="PSUM"))
        ps_p = ctx.enter_context(tc.tile_pool(name="ps_p", bufs=1, space="PSUM"))
        ps_o = ctx.enter_context(tc.tile_pool(name="ps_o", bufs=2, space="PSUM"))
        ps_s = ctx.enter_context(tc.tile_pool(name="ps_s", bufs=2, space="PSUM"))
        ps_y = ctx.enter_context(tc.tile_pool(name="ps_y", bufs=1, space="PSUM"))

        # ---------------- constants ----------------
        mask0 = const.tile([128, 128], BF16, tag="mask0")
        make_upper_triangular(nc, mask0[:], val=1.0, diag=True)
        ones_bf = const.tile([128, 1], BF16, tag="ones")
        nc.vector.memset(ones_bf[:], 1.0)
        ones_row = const.tile([1, 128], F32, tag="ones_row")
        nc.vector.memset(ones_row[:], 1.0)

        lamt = {}
        for nm in ("lq1", "lq2", "lk1", "lk2"):
            t_ = const.tile([1, HHD], F32, tag=nm)
            nc.sync.dma_start(t_[:], lam_d[nm])
            lamt[nm] = t_
        evals = []
        for a, b in (("lq1", "lk1"), ("lq2", "lk2")):
            m_ = small.tile([1, HHD], F32, tag="lmul")
            nc.vector.tensor_mul(m_[:], lamt[a][:], lamt[b][:])
            s_ = small.tile([1, 1], F32, tag="lsum")
            nc.vector.tensor_reduce(s_[:], m_[:], axis=mybir.AxisListType.X,
                                    op=mybir.AluOpType.add)
            e_ = const.tile([1, 1], F32, tag=f"e_{a}")
            nc.scalar.activation(e_[:], s_[:], AF.Exp)
            evals.append(e_)
        # neglam = -(exp(s1) - exp(s2) + LAMBDA_INIT) = e2 - e1 - LAMBDA_INIT
        neglam = const.tile([1, 1], F32, tag="neglam")
        nc.vector.tensor_sub(neglam[:], evals[1][:], evals[0][:])
        nc.vector.tensor_scalar_add(neglam[:], neglam[:], -LAMBDA_INIT)

        eps_t = const.tile([1, 1], F32, tag="eps")
        nc.vector.memset(eps_t[:], 1e-5)
        rms_t = const.tile([128, 1], F32, tag="rms")
        nc.sync.dma_start(rms_t[:], rms_d)
        woscale = const.tile([128, 1], F32, tag="wos")
        nc.vector.tensor_scalar_mul(woscale[:], rms_t[:], 1.0 - LAMBDA_INIT)

        # ---------------- weights ----------------
        wq_bf = wpool.tile([128, 8, 256], BF16, tag="wq")
        wk_bf = wpool.tile([128, 8, 256], BF16, tag="wk")
        wv_bf = wpool.tile([128, 8, 256], BF16, tag="wv")
        for wd, wt in ((wq_d, wq_bf), (wk_d, wk_bf), (wv_d, wv_bf)):
            for g in range(8):
                ws = stage.tile([128, 256], F32, tag="wstg")
                nc.sync.dma_start(ws[:], wd[128 * g:128 * (g + 1), :])
                nc.vector.tensor_copy(wt[:, g, :], ws[:])
        wo_bf = wpool.tile([128, 2, C], BF16, tag="wo")
        for h in range(2):
            ws = stage.tile([128, C], F32, tag="stg")
            nc.sync.dma_start(ws[:], wo_d[128 * h:128 * (h + 1), :])
            nc.vector.tensor_scalar_mul(ws[:], ws[:], woscale[:])
            nc.vector.tensor_copy(wo_bf[:, h, :], ws[:])

        # ---- fused per-chunk pipeline: loads/projections j overlap attention ----
        rep_ctx = tc.For_i(0, repeats, 1) if repeats > 1 else None
        if rep_ctx is not None:
            rep_ctx.__enter__()
        V_all = persist.tile([128, NT, 256], BF16, tag="V_all")
        kT_h = [persist.tile([128, T], BF16, tag=f"kT{h}", name=f"kT{h}") for h in range(2)]
        qT_h = [persist.tile([128, T], BF16, tag=f"qT{h}", name=f"qT{h}") for h in range(2)]
        for j in range(NCH):
            # value rows 512j..512(j+1) -> V_all n-tiles 4j..4j+3
            for ii in range(4):
                i = 4 * j + ii
                bst = bstage.tile([128, C], BF16, tag="bstg")
                nc.gpsimd.dma_start(bst[:], v_d[128 * i:128 * (i + 1), :])
                xv = xch.tile([128, 8, 128], BF16, tag="xsm")
                nc.sync.dma_start(xv[:], bst[:], transpose=True)
                pv = ps_p.tile([128, 512], F32, tag="pp")
                for g in range(8):
                    nc.tensor.matmul(pv[:, :256], xv[:, g, :], wv_bf[:, g, :],
                                     start=(g == 0), stop=(g == 7))
                nc.vector.tensor_copy(V_all[:, i, :], pv[:, :256])
            # key chunk j -> kT_h[:, 512j:512(j+1)]
            xk = xch.tile([128, 8, TCH], BF16, tag="xbig")
            for kk in range(4):
                gt = 4 * j + kk
                bst = bstage.tile([128, C], BF16, tag="bstg")
                nc.gpsimd.dma_start(bst[:], k_d[128 * gt:128 * (gt + 1), :])
                nc.sync.dma_start(xk[:, :, 128 * kk:128 * (kk + 1)], bst[:],
                                  transpose=True)
            for ds in range(2):
                pk = ps_p.tile([128, 512], F32, tag="pp")
                for g in range(8):
                    nc.tensor.matmul(pk[:], wk_bf[:, g, 128 * ds:128 * (ds + 1)],
                                     xk[:, g, :], start=(g == 0), stop=(g == 7))
                nc.scalar.copy(kT_h[ds][:, TCH * j:TCH * (j + 1)], pk[:])
            # query chunk j -> qT_h[:, 512j:512(j+1)]
            xq = xch.tile([128, 8, TCH], BF16, tag="xbig")
            for kk in range(4):
                gt = 4 * j + kk
                bst = bstage.tile([128, C], BF16, tag="bstg")
                nc.gpsimd.dma_start(bst[:], q_d[128 * gt:128 * (gt + 1), :])
                nc.sync.dma_start(xq[:, :, 128 * kk:128 * (kk + 1)], bst[:],
                                  transpose=True)
            for ds in range(2):
                pq = ps_p.tile([128, 512], F32, tag="pp")
                for g in range(8):
                    nc.tensor.matmul(pq[:], wq_bf[:, g, 128 * ds:128 * (ds + 1)],
                                     xq[:, g, :], start=(g == 0), stop=(g == 7))
                nc.scalar.copy(qT_h[ds][:, TCH * j:TCH * (j + 1)], pq[:])

            n_hi = 4 * j + 4
            # ---- phase A: per-head matmul blocks (scores/exp/O/rowsum) ----
            o_pss = []
            r_pss = []
            for h in range(2):
                A = {}
                for i in range(n_hi):
                    ts0 = max(0, 128 * i - TCH * j)
                    for hh in range(2):
                        r0 = 64 * hh
                        sp = ps_a.tile([128, 512], F32, tag="sps")
                        nc.tensor.matmul(
                            sp[:, ts0:], kT_h[h][r0:r0 + 64, 128 * i:128 * (i + 1)],
                            qT_h[h][r0:r0 + 64, TCH * j + ts0:TCH * (j + 1)])
                        at = apool.tile([128, 512], BF16, tag="at")
                        nc.scalar.activation(at[:, ts0:], sp[:, ts0:], AF.Exp,
                                             scale=SCALE)
                        if i >= 4 * j:
                            nc.vector.tensor_mul(at[:, ts0:ts0 + 128],
                                                 at[:, ts0:ts0 + 128], mask0[:])
                        A[(hh, i)] = (at, ts0)
                o_ps = [ps_o.tile([128, 512], F32, tag="ops", name=f"ops{_h}")
                        for _h in range(2)]
                for i in range(n_hi):
                    for hh in range(2):
                        at, ts0 = A[(hh, i)]
                        nc.tensor.matmul(o_ps[hh][:, ts0:],
                                         V_all[:, i, 128 * h:128 * (h + 1)],
                                         at[:, ts0:], start=(i == 0),
                                         stop=(i == n_hi - 1))
                # rowsums rows 0/32 = hh0/hh1; row 64 = rms square-sum
                r_ps = ps_s.tile([65, 512], F32, tag="rps")
                for i in range(n_hi):
                    for hh in range(2):
                        at, ts0 = A[(hh, i)]
                        ro = 32 * hh
                        nc.tensor.matmul(r_ps[ro:ro + 1, ts0:], ones_bf[:],
                                         at[:, ts0:], start=(i == 0),
                                         stop=(i == n_hi - 1))
                o_pss.append(o_ps)
                r_pss.append(r_ps)
            # ---- phase B: per-head softmax-division/combine/rms tails ----
            osc = []
            ocs = []
            for h in range(2):
                o_ps, r_ps = o_pss[h], r_pss[h]
                rv1 = small.tile([1, 512], F32, tag="rv1")
                nc.vector.reciprocal(rv1[:], r_ps[0:1, :])
                rv2 = small.tile([1, 512], F32, tag="rv2")
                nc.vector.reciprocal(rv2[:], r_ps[32:33, :])
                b2p = small.tile([1, 512], F32, tag="b2p")
                nc.vector.tensor_scalar_mul(b2p[:], rv2[:], neglam[:])
                bp1 = ps_y.tile([128, 512], F32, tag="ypb")
                nc.tensor.matmul(bp1[:], ones_row[:], rv1[:])
                B1 = bcast.tile([128, 512], F32, tag="B")
                nc.vector.tensor_copy(B1[:], bp1[:])
                bp2 = ps_y.tile([128, 512], F32, tag="ypb")
                nc.tensor.matmul(bp2[:], ones_row[:], b2p[:])
                B2 = bcast.tile([128, 512], F32, tag="B")
                nc.vector.tensor_copy(B2[:], bp2[:])
                t1 = work.tile([128, 512], F32, tag="wk1")
                nc.vector.tensor_mul(t1[:], o_ps[0][:], B1[:])
                t2 = work.tile([128, 512], F32, tag="wk2")
                nc.vector.tensor_mul(t2[:], o_ps[1][:], B2[:])
                oc = work.tile([128, 512], F32, tag="oc")
                nc.vector.tensor_add(oc[:], t1[:], t2[:])
                osq = work.tile([128, 512], BF16, tag="osq")
                nc.vector.tensor_mul(osq[:], oc[:], oc[:])
                nc.tensor.matmul(r_ps[64:65, :], ones_bf[:], osq[:])
                ocs.append(oc)
            for h in range(2):
                sq = small.tile([1, 512], F32, tag="sq")
                nc.scalar.activation(sq[:], r_pss[h][64:65, :], AF.Sqrt,
                                     bias=eps_t[:], scale=1.0 / DH)
                sv = small.tile([1, 512], F32, tag="sv")
                nc.vector.reciprocal(sv[:], sq[:])
                bp3 = ps_y.tile([128, 512], F32, tag="ypb")
                nc.tensor.matmul(bp3[:], ones_row[:], sv[:])
                SB = bcast.tile([128, 512], F32, tag="B")
                nc.vector.tensor_copy(SB[:], bp3[:])
                os_t = oscp.tile([128, 512], BF16, tag="osc")
                nc.vector.tensor_mul(os_t[:], ocs[h][:], SB[:])
                osc.append(os_t)
            # output projection: Y[t, :] = sum_h O_h^T(t)^T @ Wo_h
            for k4 in range(4):
                ysb = yout.tile([128, C], F32, tag="ysb")
                for half in range(2):
                    yp = ps_y.tile([128, 512], F32, tag="ypb")
                    for h in range(2):
                        lt = osc[h][:, 128 * k4:128 * (k4 + 1)]
                        nc.tensor.matmul(yp[:], lt,
                                         wo_bf[:, h, 512 * half:512 * (half + 1)],
                                         start=(h == 0), stop=(h == 1))
                    nc.vector.tensor_copy(ysb[:, 512 * half:512 * (half + 1)],
                                          yp[:])
                row = TCH * j + 128 * k4
                nc.sync.dma_start(y_d[row:row + 128, :], ysb[:])
        if rep_ctx is not None:
            rep_ctx.__exit__(None, None, None)

    nc.compile()
    return nc


_NC_CACHE = {}


def get_nc(repeats=1):
    if repeats not in _NC_CACHE:
        _NC_CACHE[repeats] = build_nc(repeats)
    return _NC_CACHE[repeats]


def make_in_maps(query, key_t, value, Wq, Wk, Wv, Wo,
                 lambda_q1, lambda_q2, lambda_k1, lambda_k2, rms_weight):
    f = np.float32
    in_maps = []
    for core in range(8):
        b, p = divmod(core, 4)
        sl = slice(256 * p, 256 * (p + 1))
        in_maps.append({
            "query": np.ascontiguousarray(query[b], dtype=f),
            "key": np.ascontiguousarray(key_t[b], dtype=f),
            "value": np.ascontiguousarray(value[b], dtype=f),
            "wq": np.ascontiguousarray(Wq[:, sl], dtype=f),
            "wk": np.ascontiguousarray(Wk[:, sl], dtype=f),
            "wv": np.ascontiguousarray(Wv[:, sl], dtype=f),
            "wo": np.ascontiguousarray(Wo[sl, :], dtype=f),
            "lq1": np.ascontiguousarray(lambda_q1, dtype=f).reshape(1, HHD),
            "lq2": np.ascontiguousarray(lambda_q2, dtype=f).reshape(1, HHD),
            "lk1": np.ascontiguousarray(lambda_k1, dtype=f).reshape(1, HHD),
            "lk2": np.ascontiguousarray(lambda_k2, dtype=f).reshape(1, HHD),
            "rmsw": np.ascontiguousarray(rms_weight, dtype=f).reshape(128, 1),
        })
    return in_maps


def kernel(query, key_t, value, Wq, Wk, Wv, Wo,
           lambda_q1, lambda_q2, lambda_k1, lambda_k2, rms_weight):
    in_maps = make_in_maps(query, key_t, value, Wq, Wk, Wv, Wo,
                           lambda_q1, lambda_q2, lambda_k1, lambda_k2,
                           rms_weight)
    res = run_bass_kernel_spmd(get_nc(), in_maps, core_ids=list(range(8)))
    parts = np.stack([res.results[i]["y"] for i in range(8)])
    return parts.reshape(2, 4, T, C).sum(axis=1).astype(np.float32)


def build_nc_baseline():
    """Same I/O signature, near-empty body — for dispatch-overhead timing."""
    nc = bacc.Bacc("TRN2", target_bir_lowering=False, debug=False,
                   enable_asserts=False)
    nc.dram_tensor("query", [T, C], F32, kind="ExternalInput")
    nc.dram_tensor("key", [T, C], F32, kind="ExternalInput")
    nc.dram_tensor("value", [T, C], F32, kind="ExternalInput")
    nc.dram_tensor("wq", [C, 256], F32, kind="ExternalInput")
    nc.dram_tensor("wk", [C, 256], F32, kind="ExternalInput")
    nc.dram_tensor("wv", [C, 256], F32, kind="ExternalInput")
    nc.dram_tensor("wo", [256, C], F32, kind="ExternalInput")
    for nm in ("lq1", "lq2", "lk1", "lk2"):
        nc.dram_tensor(nm, [1, HHD], F32, kind="ExternalInput")
    nc.dram_tensor("rmsw", [128, 1], F32, kind="ExternalInput")
    y_d = nc.dram_tensor("y", [T, C], F32, kind="ExternalOutput").ap()
    with tile.TileContext(nc) as tc, ExitStack() as ctx:
        pool = ctx.enter_context(tc.tile_pool(name="p", bufs=1))
        t_ = pool.tile([128, C], F32, tag="t")
        nc.vector.memset(t_[:], 0.0)
        nc.sync.dma_start(y_d[0:128, :], t_[:])
    nc.compile()
    return nc


# revision 17
# speedup vs baseline: 13.6932x; 1.1120x over previous
"""Differential multi-head attention (DiffMHA) Trainium2 kernel.

Full-input contract: kernel(**inputs) takes the unsharded numpy inputs and
returns the full [2, 2048, 1024] f32 output. Internally the work is sharded
across 8 NeuronCores: data-parallel over the batch (B=2) and tensor-parallel
over heads (2 value heads / 4 score half-heads per core), with Wq/Wk/Wv
column-sharded and Wo row-sharded. Each core produces a full-width partial
Y contribution; the host sums the 4 head-group partials per batch element
(the "all-reduce" of Megatron row-parallelism, done on host for free).

Per-core pipeline (all matmuls bf16 x bf16 -> f32 PSUM):
  1. inputs f32 -> SBUF, cast bf16, DMA-transpose to [c, t] layout
  2. qT/kT = Wq^T X^T ([d, t]), V = X Wv (natural [n, e])
  3. per 512-wide t-chunk: scores^T = kT^T qT per half-head (causal-skipped),
     exp on ScalarE, rowsums via ones-matmul, O^T = V^T A^T accumulation,
     deferred softmax division + differential-lambda combine on VectorE,
     RMS norm via ones-matmul of squares + partition-broadcast scale
  4. Y += O_scaled^T^T @ Wo_scaled  (rms_weight * (1-lambda_init) is folded
     into Wo rows on-chip), DMA out.
"""

import math
from contextlib import ExitStack

import numpy as np

import concourse.bacc as bacc
import concourse.mybir as mybir
import concourse.tile as tile
from concourse.bass_utils import run_bass_kernel_spmd
from concourse.masks import make_upper_triangular  # noqa: F401
from concourse.masks import make_identity

F32 = mybir.dt.float32
BF16 = mybir.dt.bfloat16
AF = mybir.ActivationFunctionType

T = 2048
C = 1024
DH = 128          # value-head dim
HHD = 64          # score half-head dim
NCH = 4           # t-chunks of 512
TCH = 512
NT = 16           # n-tiles of 128
LAMBDA_INIT = 0.8 - 0.6 * math.exp(-0.3 * 12)
SCALE = 1.0 / math.sqrt(HHD)


def build_nc(repeats=1):
    nc = bacc.Bacc("TRN2", target_bir_lowering=False, debug=False,
                   enable_asserts=False)
    q_d = nc.dram_tensor("query", [T, C], F32, kind="ExternalInput").ap()
    k_d = nc.dram_tensor("key", [T, C], F32, kind="ExternalInput").ap()
    v_d = nc.dram_tensor("value", [T, C], F32, kind="ExternalInput").ap()
    wq_d = nc.dram_tensor("wq", [C, 256], F32, kind="ExternalInput").ap()
    wk_d = nc.dram_tensor("wk", [C, 256], F32, kind="ExternalInput").ap()
    wv_d = nc.dram_tensor("wv", [C, 256], F32, kind="ExternalInput").ap()
    wo_d = nc.dram_tensor("wo", [256, C], F32, kind="ExternalInput").ap()
    lam_d = {nm: nc.dram_tensor(nm, [1, HHD], F32, kind="ExternalInput").ap()
             for nm in ("lq1", "lq2", "lk1", "lk2")}
    rms_d = nc.dram_tensor("rmsw", [128, 1], F32, kind="ExternalInput").ap()
    y_d = nc.dram_tensor("y", [T, C], F32, kind="ExternalOutput").ap()

    with tile.TileContext(nc) as tc, ExitStack() as ctx:
        const = ctx.enter_context(tc.tile_pool(name="const", bufs=1))
        wpool = ctx.enter_context(tc.tile_pool(name="wpool", bufs=1))
        stage = ctx.enter_context(tc.tile_pool(name="stage", bufs=2))
        bstage = ctx.enter_context(tc.tile_pool(name="bstage", bufs=3))
        xch = ctx.enter_context(tc.tile_pool(name="xch", bufs=2))
        persist = ctx.enter_context(tc.tile_pool(name="persist", bufs=1))
        apool = ctx.enter_context(tc.tile_pool(name="apool", bufs=36))
        small = ctx.enter_context(tc.tile_pool(name="small", bufs=2))
        bcast = ctx.enter_context(tc.tile_pool(name="bcast", bufs=4))
        work = ctx.enter_context(tc.tile_pool(name="work", bufs=2))
        oscp = ctx.enter_context(tc.tile_pool(name="oscp", bufs=3))
        yout = ctx.enter_context(tc.tile_pool(name="yout", bufs=2))
        ps_a = ctx.enter_context(tc.tile_pool(name="ps_a", bufs=2, space="PSUM"))
        ps_p = ctx.enter_context(tc.tile_pool(name="ps_p", bufs=1, space="PSUM"))
        ps_o = ctx.enter_context(tc.tile_pool(name="ps_o", bufs=2, space="PSUM"))
        ps_s = ctx.enter_context(tc.tile_pool(name="ps_s", bufs=2, space="PSUM"))
        ps_y = ctx.enter_context(tc.tile_pool(name="ps_y", bufs=1, space="PSUM"))

        # ---------------- constants ----------------
        # causal mask applied on PE: scores += (-30000*I)^T @ strict-lower-tri
        negI = const.tile([128, 128], BF16, tag="negI")
        make_identity(nc, negI[:])
        nc.vector.tensor_scalar_mul(negI[:], negI[:], -30000.0)
        ltm = const.tile([128, 128], BF16, tag="ltm")
        nc.gpsimd.memset(ltm[:], 1.0)
        nc.gpsimd.affine_select(
            out=ltm[:], in_=ltm[:], compare_op=mybir.AluOpType.is_gt,
            fill=0.0, base=0, pattern=[[-1, 128]], channel_multiplier=1)
        ones_bf = const.tile([128, 1], BF16, tag="ones")
        nc.vector.memset(ones_bf[:], 1.0)
        ones_row = const.tile([1, 128], F32, tag="ones_row")
        nc.vector.memset(ones_row[:], 1.0)

        lamt = {}
        for nm in ("lq1", "lq2", "lk1", "lk2"):
            t_ = const.tile([1, HHD], F32, tag=nm)
            nc.sync.dma_start(t_[:], lam_d[nm])
            lamt[nm] = t_
        evals = []
        for a, b in (("lq1", "lk1"), ("lq2", "lk2")):
            m_ = small.tile([1, HHD], F32, tag="lmul")
            nc.vector.tensor_mul(m_[:], lamt[a][:], lamt[b][:])
            s_ = small.tile([1, 1], F32, tag="lsum")
            nc.vector.tensor_reduce(s_[:], m_[:], axis=mybir.AxisListType.X,
                                    op=mybir.AluOpType.add)
            e_ = const.tile([1, 1], F32, tag=f"e_{a}")
            nc.scalar.activation(e_[:], s_[:], AF.Exp)
            evals.append(e_)
        # neglam = -(exp(s1) - exp(s2) + LAMBDA_INIT) = e2 - e1 - LAMBDA_INIT
        neglam = const.tile([1, 1], F32, tag="neglam")
        nc.vector.tensor_sub(neglam[:], evals[1][:], evals[0][:])
        nc.vector.tensor_scalar_add(neglam[:], neglam[:], -LAMBDA_INIT)
        neglam128 = const.tile([128, 1], F32, tag="neglam128")

        eps_t = const.tile([1, 1], F32, tag="eps")
        nc.vector.memset(eps_t[:], 1e-5)
        rms_t = const.tile([128, 1], F32, tag="rms")
        nc.sync.dma_start(rms_t[:], rms_d)
        woscale = const.tile([128, 1], F32, tag="wos")
        nc.vector.tensor_scalar_mul(woscale[:], rms_t[:], 1.0 - LAMBDA_INIT)

        # ---------------- weights ----------------
        wq_bf = wpool.tile([128, 8, 256], BF16, tag="wq")
        wk_bf = wpool.tile([128, 8, 256], BF16, tag="wk")
        wv_bf = wpool.tile([128, 8, 256], BF16, tag="wv")
        for wd, wt in ((wv_d, wv_bf), (wk_d, wk_bf), (wq_d, wq_bf)):
            for g in range(8):
                ws = stage.tile([128, 256], F32, tag="wstg", bufs=8)
                nc.sync.dma_start(ws[:], wd[128 * g:128 * (g + 1), :])
                nc.vector.tensor_copy(wt[:, g, :], ws[:])
        wo_bf = wpool.tile([128, 2, C], BF16, tag="wo")
        for h in range(2):
            ws = stage.tile([128, C], F32, tag="stg")
            nc.sync.dma_start(ws[:], wo_d[128 * h:128 * (h + 1), :])
            nc.vector.tensor_scalar_mul(ws[:], ws[:], woscale[:])
            nc.vector.tensor_copy(wo_bf[:, h, :], ws[:])

        # ---- fused per-chunk pipeline: loads/projections j overlap attention ----
        rep_ctx = tc.For_i(0, repeats, 1) if repeats > 1 else None
        if rep_ctx is not None:
            rep_ctx.__enter__()
        V_all = persist.tile([128, NT, 256], BF16, tag="V_all")
        kT_h = [persist.tile([128, T], BF16, tag=f"kT{h}", name=f"kT{h}") for h in range(2)]
        qT_h = [persist.tile([128, T], BF16, tag=f"qT{h}", name=f"qT{h}") for h in range(2)]
        for j in range(NCH):
            # value rows 512j..512(j+1) -> V_all n-tiles 4j..4j+3
            bstv = bstage.tile([128, 4, C], BF16, tag="bstg")
            nc.gpsimd.dma_start(
                bstv[:], v_d[TCH * j:TCH * (j + 1), :].rearrange(
                    "(r p) c -> p r c", p=128))
            for ii in range(4):
                i = 4 * j + ii
                xv = xch.tile([128, 8, 128], BF16, tag="xsm", bufs=6)
                nc.sync.dma_start(xv[:], bstv[:, ii, :], transpose=True)
                pv = ps_p.tile([128, 512], F32, tag="pp")
                for g in range(8):
                    nc.tensor.matmul(pv[:, :256], xv[:, g, :], wv_bf[:, g, :],
                                     start=(g == 0), stop=(g == 7))
                nc.vector.tensor_copy(V_all[:, i, :], pv[:, :256])
            # key chunk j -> kT_h[:, 512j:512(j+1)]
            xk = xch.tile([128, 8, TCH], BF16, tag="xbig", bufs=3)
            bstk = bstage.tile([128, 4, C], BF16, tag="bstg")
            nc.gpsimd.dma_start(
                bstk[:], k_d[TCH * j:TCH * (j + 1), :].rearrange(
                    "(r p) c -> p r c", p=128))
            for kk in range(4):
                nc.sync.dma_start(xk[:, :, 128 * kk:128 * (kk + 1)],
                                  bstk[:, kk, :], transpose=True)
            for ds in range(2):
                pk = ps_p.tile([128, 512], F32, tag="pp")
                for g in range(8):
                    nc.tensor.matmul(pk[:], wk_bf[:, g, 128 * ds:128 * (ds + 1)],
                                     xk[:, g, :], start=(g == 0), stop=(g == 7))
                nc.scalar.copy(kT_h[ds][:, TCH * j:TCH * (j + 1)], pk[:])
            # query chunk j -> qT_h[:, 512j:512(j+1)]
            xq = xch.tile([128, 8, TCH], BF16, tag="xbig", bufs=3)
            bstq = bstage.tile([128, 4, C], BF16, tag="bstg")
            nc.gpsimd.dma_start(
                bstq[:], q_d[TCH * j:TCH * (j + 1), :].rearrange(
                    "(r p) c -> p r c", p=128))
            for kk in range(4):
                nc.sync.dma_start(xq[:, :, 128 * kk:128 * (kk + 1)],
                                  bstq[:, kk, :], transpose=True)
            for ds in range(2):
                pq = ps_p.tile([128, 512], F32, tag="pp")
                for g in range(8):
                    nc.tensor.matmul(pq[:], wq_bf[:, g, 128 * ds:128 * (ds + 1)],
                                     xq[:, g, :], start=(g == 0), stop=(g == 7))
                nc.scalar.copy(qT_h[ds][:, TCH * j:TCH * (j + 1)], pq[:])

            n_hi = 4 * j + 4
            # ---- phase A: per-head matmul blocks (scores/exp/O/rowsum) ----
            o_pss = []
            r_pss = []
            for h in range(2):
                A = {}
                for i in range(n_hi):
                    ts0 = max(0, 128 * i - TCH * j)
                    for hh in range(2):
                        r0 = 64 * hh
                        diag = i >= 4 * j
                        sp = ps_a.tile([128, 512], F32, tag="sps")
                        nc.tensor.matmul(
                            sp[:, ts0:], kT_h[h][r0:r0 + 64, 128 * i:128 * (i + 1)],
                            qT_h[h][r0:r0 + 64, TCH * j + ts0:TCH * (j + 1)],
                            start=True, stop=not diag)
                        if diag:
                            nc.tensor.matmul(sp[:, ts0:ts0 + 128], negI[:],
                                             ltm[:], start=False, stop=True)
                        at = apool.tile([128, 512], BF16, tag="at")
                        nc.scalar.activation(at[:, ts0:], sp[:, ts0:], AF.Exp,
                                             scale=SCALE)
                        A[(hh, i)] = (at, ts0)
                o_ps = [ps_o.tile([128, 512], F32, tag="ops", name=f"ops{_h}")
                        for _h in range(2)]
                for i in range(n_hi):
                    for hh in range(2):
                        at, ts0 = A[(hh, i)]
                        nc.tensor.matmul(o_ps[hh][:, ts0:],
                                         V_all[:, i, 128 * h:128 * (h + 1)],
                                         at[:, ts0:], start=(i == 0),
                                         stop=(i == n_hi - 1))
                # rowsums rows 0/32 = hh0/hh1; row 64 = rms square-sum
                r_ps = ps_s.tile([65, 512], F32, tag="rps")
                for i in range(n_hi):
                    for hh in range(2):
                        at, ts0 = A[(hh, i)]
                        ro = 32 * hh
                        nc.tensor.matmul(r_ps[ro:ro + 1, ts0:], ones_bf[:],
                                         at[:, ts0:], start=(i == 0),
                                         stop=(i == n_hi - 1))
                o_pss.append(o_ps)
                r_pss.append(r_ps)
            # ---- phase B: per-head softmax-division/combine/rms tails ----
            osc = []
            ocs = []
            for h in range(2):
                if j == 0 and h == 0:
                    nc.gpsimd.partition_broadcast(neglam128[:], neglam[:])
                o_ps, r_ps = o_pss[h], r_pss[h]
                rv1 = small.tile([1, 512], F32, tag="rv1")
                nc.vector.reciprocal(rv1[:], r_ps[0:1, :])
                rv2 = small.tile([1, 512], F32, tag="rv2")
                nc.vector.reciprocal(rv2[:], r_ps[32:33, :])
                bp1 = ps_y.tile([128, 512], F32, tag="ypb")
                nc.tensor.matmul(bp1[:], ones_row[:], rv1[:])
                B1 = bcast.tile([128, 512], F32, tag="B")
                nc.vector.tensor_copy(B1[:], bp1[:])
                bp2 = ps_y.tile([128, 512], F32, tag="ypb")
                nc.tensor.matmul(bp2[:], ones_row[:], rv2[:])
                B2 = bcast.tile([128, 512], F32, tag="B")
                nc.vector.tensor_copy(B2[:], bp2[:])
                t1 = work.tile([128, 512], F32, tag="wk1")
                nc.vector.tensor_mul(t1[:], o_ps[0][:], B1[:])
                t2 = work.tile([128, 512], F32, tag="wk2")
                nc.vector.scalar_tensor_tensor(
                    t2[:], o_ps[1][:], neglam128[:], B2[:],
                    op0=mybir.AluOpType.mult, op1=mybir.AluOpType.mult)
                oc = work.tile([128, 512], F32, tag="oc")
                nc.vector.tensor_add(oc[:], t1[:], t2[:])
                osq = work.tile([128, 512], BF16, tag="osq")
                nc.vector.tensor_mul(osq[:], oc[:], oc[:])
                nc.tensor.matmul(r_ps[64:65, :], ones_bf[:], osq[:])
                ocs.append(oc)
            for h in range(2):
                sq = small.tile([1, 512], F32, tag="sq")
                nc.scalar.activation(sq[:], r_pss[h][64:65, :], AF.Sqrt,
                                     bias=eps_t[:], scale=1.0 / DH)
                sv = small.tile([1, 512], F32, tag="sv")
                nc.vector.reciprocal(sv[:], sq[:])
                bp3 = ps_y.tile([128, 512], F32, tag="ypb")
                nc.tensor.matmul(bp3[:], ones_row[:], sv[:])
                SB = bcast.tile([128, 512], F32, tag="B")
                nc.vector.tensor_copy(SB[:], bp3[:])
                os_t = oscp.tile([128, 512], BF16, tag="osc")
                nc.vector.tensor_mul(os_t[:], ocs[h][:], SB[:])
                osc.append(os_t)
            # output projection: Y[t, :] = sum_h O_h^T(t)^T @ Wo_h
            for k4 in range(4):
                ysb = yout.tile([128, C], F32, tag="ysb")
                for half in range(2):
                    yp = ps_y.tile([128, 512], F32, tag="ypb")
                    for h in range(2):
                        lt = osc[h][:, 128 * k4:128 * (k4 + 1)]
                        nc.tensor.matmul(yp[:], lt,
                                         wo_bf[:, h, 512 * half:512 * (half + 1)],
                                         start=(h == 0), stop=(h == 1))
                    nc.vector.tensor_copy(ysb[:, 512 * half:512 * (half + 1)],
                                          yp[:])
                row = TCH * j + 128 * k4
                nc.sync.dma_start(y_d[row:row + 128, :], ysb[:])
        if rep_ctx is not None:
            rep_ctx.__exit__(None, None, None)

    nc.compile()
    return nc


_NC_CACHE = {}


def get_nc(repeats=1):
    if repeats not in _NC_CACHE:
        _NC_CACHE[repeats] = build_nc(repeats)
    return _NC_CACHE[repeats]


def make_in_maps(query, key_t, value, Wq, Wk, Wv, Wo,
                 lambda_q1, lambda_q2, lambda_k1, lambda_k2, rms_weight):
    f = np.float32
    in_maps = []
    for core in range(8):
        b, p = divmod(core, 4)
        sl = slice(256 * p, 256 * (p + 1))
        in_maps.append({
            "query": np.ascontiguousarray(query[b], dtype=f),
            "key": np.ascontiguousarray(key_t[b], dtype=f),
            "value": np.ascontiguousarray(value[b], dtype=f),
            "wq": np.ascontiguousarray(Wq[:, sl], dtype=f),
            "wk": np.ascontiguousarray(Wk[:, sl], dtype=f),
            "wv": np.ascontiguousarray(Wv[:, sl], dtype=f),
            "wo": np.ascontiguousarray(Wo[sl, :], dtype=f),
            "lq1": np.ascontiguousarray(lambda_q1, dtype=f).reshape(1, HHD),
            "lq2": np.ascontiguousarray(lambda_q2, dtype=f).reshape(1, HHD),
            "lk1": np.ascontiguousarray(lambda_k1, dtype=f).reshape(1, HHD),
            "lk2": np.ascontiguousarray(lambda_k2, dtype=f).reshape(1, HHD),
            "rmsw": np.ascontiguousarray(rms_weight, dtype=f).reshape(128, 1),
        })
    return in_maps


def kernel(query, key_t, value, Wq, Wk, Wv, Wo,
           lambda_q1, lambda_q2, lambda_k1, lambda_k2, rms_weight):
    in_maps = make_in_maps(query, key_t, value, Wq, Wk, Wv, Wo,
                           lambda_q1, lambda_q2, lambda_k1, lambda_k2,
                           rms_weight)
    res = run_bass_kernel_spmd(get_nc(), in_maps, core_ids=list(range(8)))
    parts = np.stack([res.results[i]["y"] for i in range(8)])
    return parts.reshape(2, 4, T, C).sum(axis=1).astype(np.float32)


def build_nc_baseline():
    """Same I/O signature, near-empty body — for dispatch-overhead timing."""
    nc = bacc.Bacc("TRN2", target_bir_lowering=False, debug=False,
                   enable_asserts=False)
    nc.dram_tensor("query", [T, C], F32, kind="ExternalInput")
    nc.dram_tensor("key", [T, C], F32, kind="ExternalInput")
    nc.dram_tensor("value", [T, C], F32, kind="ExternalInput")
    nc.dram_tensor("wq", [C, 256], F32, kind="ExternalInput")
    nc.dram_tensor("wk", [C, 256], F32, kind="ExternalInput")
    nc.dram_tensor("wv", [C, 256], F32, kind="ExternalInput")
    nc.dram_tensor("wo", [256, C], F32, kind="ExternalInput")
    for nm in ("lq1", "lq2", "lk1", "lk2"):
        nc.dram_tensor(nm, [1, HHD], F32, kind="ExternalInput")
    nc.dram_tensor("rmsw", [128, 1], F32, kind="ExternalInput")
    y_d = nc.dram_tensor("y", [T, C], F32, kind="ExternalOutput").ap()
    with tile.TileContext(nc) as tc, ExitStack() as ctx:
        pool = ctx.enter_context(tc.tile_pool(name="p", bufs=1))
        t_ = pool.tile([128, C], F32, tag="t")
        nc.vector.memset(t_[:], 0.0)
        nc.sync.dma_start(y_d[0:128, :], t_[:])
    nc.compile()
    return nc


# revision 20
# speedup vs baseline: 15.3530x; 1.1212x over previous
"""Differential multi-head attention (DiffMHA) Trainium2 kernel.

Full-input contract: kernel(**inputs) takes the unsharded numpy inputs and
returns the full [2, 2048, 1024] f32 output. Internally the work is sharded
across 8 NeuronCores: data-parallel over the batch (B=2) and tensor-parallel
over heads (2 value heads / 4 score half-heads per core), with Wq/Wk/Wv
column-sharded and Wo row-sharded. Each core produces a full-width partial
Y contribution; the host sums the 4 head-group partials per batch element
(the "all-reduce" of Megatron row-parallelism, done on host for free).

Per-core pipeline (all matmuls bf16 x bf16 -> f32 PSUM):
  1. inputs f32 -> SBUF, cast bf16, DMA-transpose to [c, t] layout
  2. qT/kT = Wq^T X^T ([d, t]), V = X Wv (natural [n, e])
  3. per 512-wide t-chunk: scores^T = kT^T qT per half-head (causal-skipped),
     exp on ScalarE, rowsums via ones-matmul, O^T = V^T A^T accumulation,
     deferred softmax division + differential-lambda combine on VectorE,
     RMS norm via ones-matmul of squares + partition-broadcast scale
  4. Y += O_scaled^T^T @ Wo_scaled  (rms_weight * (1-lambda_init) is folded
     into Wo rows on-chip), DMA out.
"""

import math
from contextlib import ExitStack

import numpy as np

import concourse.bacc as bacc
import concourse.mybir as mybir
import concourse.tile as tile
from concourse.bass_utils import run_bass_kernel_spmd
from concourse.masks import make_upper_triangular  # noqa: F401
from concourse.masks import make_identity

F32 = mybir.dt.float32
BF16 = mybir.dt.bfloat16
AF = mybir.ActivationFunctionType

T = 2048
C = 1024
DH = 128          # value-head dim
HHD = 64          # score half-head dim
NCH = 4           # t-chunks of 512
TCH = 512
NT = 16           # n-tiles of 128
LAMBDA_INIT = 0.8 - 0.6 * math.exp(-0.3 * 12)
SCALE = 1.0 / math.sqrt(HHD)


def build_nc(repeats=1):
    nc = bacc.Bacc("TRN2", target_bir_lowering=False, debug=False,
                   enable_asserts=False)
    q_d = nc.dram_tensor("query", [T, C], F32, kind="ExternalInput").ap()
    k_d = nc.dram_tensor("key", [T, C], F32, kind="ExternalInput").ap()
    v_d = nc.dram_tensor("value", [T, C], F32, kind="ExternalInput").ap()
    wq_d = nc.dram_tensor("wq", [C, 256], F32, kind="ExternalInput").ap()
    wk_d = nc.dram_tensor("wk", [C, 256], F32, kind="ExternalInput").ap()
    wv_d = nc.dram_tensor("wv", [C, 256], F32, kind="ExternalInput").ap()
    wo_d = nc.dram_tensor("wo", [256, C], F32, kind="ExternalInput").ap()
    lam_d = {nm: nc.dram_tensor(nm, [1, HHD], F32, kind="ExternalInput").ap()
             for nm in ("lq1", "lq2", "lk1", "lk2")}
    rms_d = nc.dram_tensor("rmsw", [128, 1], F32, kind="ExternalInput").ap()
    y_d = nc.dram_tensor("y", [T, C], F32, kind="ExternalOutput").ap()

    with tile.TileContext(nc) as tc, ExitStack() as ctx:
        const = ctx.enter_context(tc.tile_pool(name="const", bufs=1))
        wpool = ctx.enter_context(tc.tile_pool(name="wpool", bufs=1))
        stage = ctx.enter_context(tc.tile_pool(name="stage", bufs=2))
        bstage = ctx.enter_context(tc.tile_pool(name="bstage", bufs=3))
        xch = ctx.enter_context(tc.tile_pool(name="xch", bufs=2))
        persist = ctx.enter_context(tc.tile_pool(name="persist", bufs=1))
        apool = ctx.enter_context(tc.tile_pool(name="apool", bufs=36))
        small = ctx.enter_context(tc.tile_pool(name="small", bufs=2))
        bcast = ctx.enter_context(tc.tile_pool(name="bcast", bufs=4))
        work = ctx.enter_context(tc.tile_pool(name="work", bufs=2))
        oscp = ctx.enter_context(tc.tile_pool(name="oscp", bufs=3))
        yout = ctx.enter_context(tc.tile_pool(name="yout", bufs=2))
        ps_a = ctx.enter_context(tc.tile_pool(name="ps_a", bufs=2, space="PSUM"))
        ps_p = ctx.enter_context(tc.tile_pool(name="ps_p", bufs=1, space="PSUM"))
        ps_o = ctx.enter_context(tc.tile_pool(name="ps_o", bufs=2, space="PSUM"))
        ps_s = ctx.enter_context(tc.tile_pool(name="ps_s", bufs=2, space="PSUM"))
        ps_y = ctx.enter_context(tc.tile_pool(name="ps_y", bufs=1, space="PSUM"))

        # ---------------- constants ----------------
        # causal mask applied on PE: scores += (-30000*I)^T @ strict-lower-tri
        negI = const.tile([128, 128], BF16, tag="negI")
        make_identity(nc, negI[:])
        nc.vector.tensor_scalar_mul(negI[:], negI[:], -30000.0)
        ltm = const.tile([128, 128], BF16, tag="ltm")
        nc.gpsimd.memset(ltm[:], 1.0)
        nc.gpsimd.affine_select(
            out=ltm[:], in_=ltm[:], compare_op=mybir.AluOpType.is_gt,
            fill=0.0, base=0, pattern=[[-1, 128]], channel_multiplier=1)
        ones_bf = const.tile([128, 1], BF16, tag="ones")
        nc.vector.memset(ones_bf[:], 1.0)
        ones_row = const.tile([1, 128], F32, tag="ones_row")
        nc.vector.memset(ones_row[:], 1.0)

        lamt = {}
        for nm in ("lq1", "lq2", "lk1", "lk2"):
            t_ = const.tile([1, HHD], F32, tag=nm)
            nc.sync.dma_start(t_[:], lam_d[nm])
            lamt[nm] = t_
        evals = []
        for a, b in (("lq1", "lk1"), ("lq2", "lk2")):
            m_ = small.tile([1, HHD], F32, tag="lmul")
            nc.vector.tensor_mul(m_[:], lamt[a][:], lamt[b][:])
            s_ = small.tile([1, 1], F32, tag="lsum")
            nc.vector.tensor_reduce(s_[:], m_[:], axis=mybir.AxisListType.X,
                                    op=mybir.AluOpType.add)
            e_ = const.tile([1, 1], F32, tag=f"e_{a}")
            nc.scalar.activation(e_[:], s_[:], AF.Exp)
            evals.append(e_)
        # neglam = -(exp(s1) - exp(s2) + LAMBDA_INIT) = e2 - e1 - LAMBDA_INIT
        neglam = const.tile([1, 1], F32, tag="neglam")
        nc.vector.tensor_sub(neglam[:], evals[1][:], evals[0][:])
        nc.vector.tensor_scalar_add(neglam[:], neglam[:], -LAMBDA_INIT)
        neglam128 = const.tile([128, 1], F32, tag="neglam128")

        eps_t = const.tile([1, 1], F32, tag="eps")
        nc.vector.memset(eps_t[:], 1e-5)
        rms_t = const.tile([128, 1], F32, tag="rms")
        nc.sync.dma_start(rms_t[:], rms_d)
        woscale = const.tile([128, 1], F32, tag="wos")
        nc.vector.tensor_scalar_mul(woscale[:], rms_t[:], 1.0 - LAMBDA_INIT)

        # ---------------- weights ----------------
        wq_bf = wpool.tile([128, 8, 256], BF16, tag="wq")
        wk_bf = wpool.tile([128, 8, 256], BF16, tag="wk")
        wv_bf = wpool.tile([128, 8, 256], BF16, tag="wv")
        for wd, wt in ((wv_d, wv_bf), (wk_d, wk_bf), (wq_d, wq_bf)):
            for g2 in range(4):
                ws = stage.tile([128, 2, 256], F32, tag="wstg", bufs=6)
                nc.sync.dma_start(
                    ws[:], wd[256 * g2:256 * (g2 + 1), :].rearrange(
                        "(r p) c -> p r c", p=128))
                nc.vector.tensor_copy(wt[:, 2 * g2:2 * (g2 + 1), :], ws[:])
        wo_bf = wpool.tile([128, 2, C], BF16, tag="wo")
        for h in range(2):
            ws = stage.tile([128, C], F32, tag="stg")
            nc.sync.dma_start(ws[:], wo_d[128 * h:128 * (h + 1), :])
            nc.vector.tensor_scalar_mul(ws[:], ws[:], woscale[:])
            nc.vector.tensor_copy(wo_bf[:, h, :], ws[:])

        # ---- fused per-chunk pipeline: loads/projections j overlap attention ----
        rep_ctx = tc.For_i(0, repeats, 1) if repeats > 1 else None
        if rep_ctx is not None:
            rep_ctx.__enter__()
        V_all = persist.tile([128, NT, 256], BF16, tag="V_all")
        kT_h = [persist.tile([128, T], BF16, tag=f"kT{h}", name=f"kT{h}") for h in range(2)]
        qT_h = [persist.tile([128, T], BF16, tag=f"qT{h}", name=f"qT{h}") for h in range(2)]

        def emit_loads(j):
            # value rows 512j..512(j+1) -> V_all n-tiles 4j..4j+3
            bstv = bstage.tile([128, 4, C], BF16, tag="bstg", name="bstv")
            nc.gpsimd.dma_start(
                bstv[:], v_d[TCH * j:TCH * (j + 1), :].rearrange(
                    "(r p) c -> p r c", p=128))
            for ii in range(4):
                i = 4 * j + ii
                xv = xch.tile([128, 8, 128], BF16, tag="xsm", bufs=6, name="xv")
                nc.sync.dma_start(xv[:], bstv[:, ii, :], transpose=True)
                pv = ps_p.tile([128, 512], F32, tag="pp", name="pv")
                for g in range(8):
                    nc.tensor.matmul(pv[:, :256], xv[:, g, :], wv_bf[:, g, :],
                                     start=(g == 0), stop=(g == 7))
                nc.vector.tensor_copy(V_all[:, i, :], pv[:, :256])
            # key chunk j -> kT_h[:, 512j:512(j+1)]
            xk = xch.tile([128, 8, TCH], BF16, tag="xbig", bufs=3, name="xk")
            bstk = bstage.tile([128, 4, C], BF16, tag="bstg", name="bstk")
            nc.gpsimd.dma_start(
                bstk[:], k_d[TCH * j:TCH * (j + 1), :].rearrange(
                    "(r p) c -> p r c", p=128))
            for kk in range(4):
                nc.sync.dma_start(xk[:, :, 128 * kk:128 * (kk + 1)],
                                  bstk[:, kk, :], transpose=True)
            for ds in range(2):
                pk = ps_p.tile([128, 512], F32, tag="pp", name="pk")
                for g in range(8):
                    nc.tensor.matmul(pk[:], wk_bf[:, g, 128 * ds:128 * (ds + 1)],
                                     xk[:, g, :], start=(g == 0), stop=(g == 7))
                nc.vector.tensor_copy(kT_h[ds][:, TCH * j:TCH * (j + 1)], pk[:])
            # query chunk j -> qT_h[:, 512j:512(j+1)]
            xq = xch.tile([128, 8, TCH], BF16, tag="xbig", bufs=3, name="xq")
            bstq = bstage.tile([128, 4, C], BF16, tag="bstg", name="bstq")
            nc.gpsimd.dma_start(
                bstq[:], q_d[TCH * j:TCH * (j + 1), :].rearrange(
                    "(r p) c -> p r c", p=128))
            for kk in range(4):
                nc.sync.dma_start(xq[:, :, 128 * kk:128 * (kk + 1)],
                                  bstq[:, kk, :], transpose=True)
            for ds in range(2):
                pq = ps_p.tile([128, 512], F32, tag="pp", name="pq")
                for g in range(8):
                    nc.tensor.matmul(pq[:], wq_bf[:, g, 128 * ds:128 * (ds + 1)],
                                     xq[:, g, :], start=(g == 0), stop=(g == 7))
                nc.vector.tensor_copy(qT_h[ds][:, TCH * j:TCH * (j + 1)], pq[:])

        for j in range(NCH):
            if j == 0:
                emit_loads(0)
            n_hi = 4 * j + 4
            # ---- phase A: per-head matmul blocks (scores/exp/O/rowsum) ----
            o_pss = []
            r_pss = []
            for h in range(2):
                A = {}
                for i in range(n_hi):
                    ts0 = max(0, 128 * i - TCH * j)
                    for hh in range(2):
                        r0 = 64 * hh
                        diag = i >= 4 * j
                        sp = ps_a.tile([128, 512], F32, tag="sps")
                        nc.tensor.matmul(
                            sp[:, ts0:], kT_h[h][r0:r0 + 64, 128 * i:128 * (i + 1)],
                            qT_h[h][r0:r0 + 64, TCH * j + ts0:TCH * (j + 1)],
                            start=True, stop=not diag)
                        if diag:
                            nc.tensor.matmul(sp[:, ts0:ts0 + 128], negI[:],
                                             ltm[:], start=False, stop=True)
                        at = apool.tile([128, 512], BF16, tag="at")
                        nc.scalar.activation(at[:, ts0:], sp[:, ts0:], AF.Exp,
                                             scale=SCALE)
                        A[(hh, i)] = (at, ts0)
                o_ps = [ps_o.tile([128, 512], F32, tag="ops", name=f"ops{_h}")
                        for _h in range(2)]
                for i in range(n_hi):
                    for hh in range(2):
                        at, ts0 = A[(hh, i)]
                        nc.tensor.matmul(o_ps[hh][:, ts0:],
                                         V_all[:, i, 128 * h:128 * (h + 1)],
                                         at[:, ts0:], start=(i == 0),
                                         stop=(i == n_hi - 1))
                # rowsums rows 0/32 = hh0/hh1; row 64 = rms square-sum
                r_ps = ps_s.tile([65, 512], F32, tag="rps")
                for i in range(n_hi):
                    for hh in range(2):
                        at, ts0 = A[(hh, i)]
                        ro = 32 * hh
                        nc.tensor.matmul(r_ps[ro:ro + 1, ts0:], ones_bf[:],
                                         at[:, ts0:], start=(i == 0),
                                         stop=(i == n_hi - 1))
                o_pss.append(o_ps)
                r_pss.append(r_ps)
            if j + 1 < NCH:
                emit_loads(j + 1)
            # ---- phase B: per-head softmax-division/combine/rms tails ----
            osc = []
            ocs = []
            for h in range(2):
                if j == 0 and h == 0:
                    nc.gpsimd.partition_broadcast(neglam128[:], neglam[:])
                o_ps, r_ps = o_pss[h], r_pss[h]
                rv1 = small.tile([1, 512], F32, tag="rv1")
                nc.vector.reciprocal(rv1[:], r_ps[0:1, :])
                rv2 = small.tile([1, 512], F32, tag="rv2")
                nc.vector.reciprocal(rv2[:], r_ps[32:33, :])
                bp1 = ps_y.tile([128, 512], F32, tag="ypb")
                nc.tensor.matmul(bp1[:], ones_row[:], rv1[:])
                B1 = bcast.tile([128, 512], F32, tag="B")
                nc.vector.tensor_copy(B1[:], bp1[:])
                bp2 = ps_y.tile([128, 512], F32, tag="ypb")
                nc.tensor.matmul(bp2[:], ones_row[:], rv2[:])
                B2 = bcast.tile([128, 512], F32, tag="B")
                nc.vector.tensor_copy(B2[:], bp2[:])
                t1 = work.tile([128, 512], F32, tag="wk1")
                nc.vector.tensor_mul(t1[:], o_ps[0][:], B1[:])
                t2 = work.tile([128, 512], F32, tag="wk2")
                nc.vector.scalar_tensor_tensor(
                    t2[:], o_ps[1][:], neglam128[:], B2[:],
                    op0=mybir.AluOpType.mult, op1=mybir.AluOpType.mult)
                oc = work.tile([128, 512], F32, tag="oc")
                nc.vector.tensor_add(oc[:], t1[:], t2[:])
                osq = work.tile([128, 512], BF16, tag="osq")
                nc.vector.tensor_mul(osq[:], oc[:], oc[:])
                nc.tensor.matmul(r_ps[64:65, :], ones_bf[:], osq[:])
                ocs.append(oc)
            for h in range(2):
                sq = small.tile([1, 512], F32, tag="sq")
                nc.scalar.activation(sq[:], r_pss[h][64:65, :], AF.Sqrt,
                                     bias=eps_t[:], scale=1.0 / DH)
                sv = small.tile([1, 512], F32, tag="sv")
                nc.vector.reciprocal(sv[:], sq[:])
                bp3 = ps_y.tile([128, 512], F32, tag="ypb")
                nc.tensor.matmul(bp3[:], ones_row[:], sv[:])
                SB = bcast.tile([128, 512], F32, tag="B")
                nc.vector.tensor_copy(SB[:], bp3[:])
                os_t = oscp.tile([128, 512], BF16, tag="osc")
                nc.vector.tensor_mul(os_t[:], ocs[h][:], SB[:])
                osc.append(os_t)
            # output projection: Y[t, :] = sum_h O_h^T(t)^T @ Wo_h
            for k4 in range(4):
                ysb = yout.tile([128, C], F32, tag="ysb")
                for half in range(2):
                    yp = ps_y.tile([128, 512], F32, tag="ypb")
                    for h in range(2):
                        lt = osc[h][:, 128 * k4:128 * (k4 + 1)]
                        nc.tensor.matmul(yp[:], lt,
                                         wo_bf[:, h, 512 * half:512 * (half + 1)],
                                         start=(h == 0), stop=(h == 1))
                    eng = nc.vector if (k4 + half) % 2 else nc.scalar
                    if eng is nc.vector:
                        nc.vector.tensor_copy(
                            ysb[:, 512 * half:512 * (half + 1)], yp[:])
                    else:
                        nc.scalar.copy(ysb[:, 512 * half:512 * (half + 1)], yp[:])
                row = TCH * j + 128 * k4
                nc.sync.dma_start(y_d[row:row + 128, :], ysb[:])
        if rep_ctx is not None:
            rep_ctx.__exit__(None, None, None)

    nc.compile()
    return nc


_NC_CACHE = {}


def get_nc(repeats=1):
    if repeats not in _NC_CACHE:
        _NC_CACHE[repeats] = build_nc(repeats)
    return _NC_CACHE[repeats]


def make_in_maps(query, key_t, value, Wq, Wk, Wv, Wo,
                 lambda_q1, lambda_q2, lambda_k1, lambda_k2, rms_weight):
    f = np.float32
    in_maps = []
    for core in range(8):
        b, p = divmod(core, 4)
        sl = slice(256 * p, 256 * (p + 1))
        in_maps.append({
            "query": np.ascontiguousarray(query[b], dtype=f),
            "key": np.ascontiguousarray(key_t[b], dtype=f),
            "value": np.ascontiguousarray(value[b], dtype=f),
            "wq": np.ascontiguousarray(Wq[:, sl], dtype=f),
            "wk": np.ascontiguousarray(Wk[:, sl], dtype=f),
            "wv": np.ascontiguousarray(Wv[:, sl], dtype=f),
            "wo": np.ascontiguousarray(Wo[sl, :], dtype=f),
            "lq1": np.ascontiguousarray(lambda_q1, dtype=f).reshape(1, HHD),
            "lq2": np.ascontiguousarray(lambda_q2, dtype=f).reshape(1, HHD),
            "lk1": np.ascontiguousarray(lambda_k1, dtype=f).reshape(1, HHD),
            "lk2": np.ascontiguousarray(lambda_k2, dtype=f).reshape(1, HHD),
            "rmsw": np.ascontiguousarray(rms_weight, dtype=f).reshape(128, 1),
        })
    return in_maps


def kernel(query, key_t, value, Wq, Wk, Wv, Wo,
           lambda_q1, lambda_q2, lambda_k1, lambda_k2, rms_weight):
    in_maps = make_in_maps(query, key_t, value, Wq, Wk, Wv, Wo,
                           lambda_q1, lambda_q2, lambda_k1, lambda_k2,
                           rms_weight)
    res = run_bass_kernel_spmd(get_nc(), in_maps, core_ids=list(range(8)))
    parts = np.stack([res.results[i]["y"] for i in range(8)])
    return parts.reshape(2, 4, T, C).sum(axis=1).astype(np.float32)


def build_nc_baseline():
    """Same I/O signature, near-empty body — for dispatch-overhead timing."""
    nc = bacc.Bacc("TRN2", target_bir_lowering=False, debug=False,
                   enable_asserts=False)
    nc.dram_tensor("query", [T, C], F32, kind="ExternalInput")
    nc.dram_tensor("key", [T, C], F32, kind="ExternalInput")
    nc.dram_tensor("value", [T, C], F32, kind="ExternalInput")
    nc.dram_tensor("wq", [C, 256], F32, kind="ExternalInput")
    nc.dram_tensor("wk", [C, 256], F32, kind="ExternalInput")
    nc.dram_tensor("wv", [C, 256], F32, kind="ExternalInput")
    nc.dram_tensor("wo", [256, C], F32, kind="ExternalInput")
    for nm in ("lq1", "lq2", "lk1", "lk2"):
        nc.dram_tensor(nm, [1, HHD], F32, kind="ExternalInput")
    nc.dram_tensor("rmsw", [128, 1], F32, kind="ExternalInput")
    y_d = nc.dram_tensor("y", [T, C], F32, kind="ExternalOutput").ap()
    with tile.TileContext(nc) as tc, ExitStack() as ctx:
        pool = ctx.enter_context(tc.tile_pool(name="p", bufs=1))
        t_ = pool.tile([128, C], F32, tag="t")
        nc.vector.memset(t_[:], 0.0)
        nc.sync.dma_start(y_d[0:128, :], t_[:])
    nc.compile()
    return nc


# revision 21
# speedup vs baseline: 15.8031x; 1.0293x over previous
"""Differential multi-head attention (DiffMHA) Trainium2 kernel.

Full-input contract: kernel(**inputs) takes the unsharded numpy inputs and
returns the full [2, 2048, 1024] f32 output. Internally the work is sharded
across 8 NeuronCores: data-parallel over the batch (B=2) and tensor-parallel
over heads (2 value heads / 4 score half-heads per core), with Wq/Wk/Wv
column-sharded and Wo row-sharded. Each core produces a full-width partial
Y contribution; the host sums the 4 head-group partials per batch element
(the "all-reduce" of Megatron row-parallelism, done on host for free).

Per-core pipeline (all matmuls bf16 x bf16 -> f32 PSUM):
  1. inputs f32 -> SBUF, cast bf16, DMA-transpose to [c, t] layout
  2. qT/kT = Wq^T X^T ([d, t]), V = X Wv (natural [n, e])
  3. per 512-wide t-chunk: scores^T = kT^T qT per half-head (causal-skipped),
     exp on ScalarE, rowsums via ones-matmul, O^T = V^T A^T accumulation,
     deferred softmax division + differential-lambda combine on VectorE,
     RMS norm via ones-matmul of squares + partition-broadcast scale
  4. Y += O_scaled^T^T @ Wo_scaled  (rms_weight * (1-lambda_init) is folded
     into Wo rows on-chip), DMA out.
"""

import math
from contextlib import ExitStack

import numpy as np

import concourse.bacc as bacc
import concourse.mybir as mybir
import concourse.tile as tile
from concourse.bass_utils import run_bass_kernel_spmd
from concourse.masks import make_upper_triangular  # noqa: F401
from concourse.masks import make_identity

F32 = mybir.dt.float32
BF16 = mybir.dt.bfloat16
AF = mybir.ActivationFunctionType

T = 2048
C = 1024
DH = 128          # value-head dim
HHD = 64          # score half-head dim
NCH = 4           # t-chunks of 512
TCH = 512
NT = 16           # n-tiles of 128
LAMBDA_INIT = 0.8 - 0.6 * math.exp(-0.3 * 12)
SCALE = 1.0 / math.sqrt(HHD)


def build_nc(repeats=1):
    nc = bacc.Bacc("TRN2", target_bir_lowering=False, debug=False,
                   enable_asserts=False)
    q_d = nc.dram_tensor("query", [T, C], F32, kind="ExternalInput").ap()
    k_d = nc.dram_tensor("key", [T, C], F32, kind="ExternalInput").ap()
    v_d = nc.dram_tensor("value", [T, C], F32, kind="ExternalInput").ap()
    wq_d = nc.dram_tensor("wq", [C, 256], F32, kind="ExternalInput").ap()
    wk_d = nc.dram_tensor("wk", [C, 256], F32, kind="ExternalInput").ap()
    wv_d = nc.dram_tensor("wv", [C, 256], F32, kind="ExternalInput").ap()
    wo_d = nc.dram_tensor("wo", [256, C], F32, kind="ExternalInput").ap()
    lam_d = {nm: nc.dram_tensor(nm, [1, HHD], F32, kind="ExternalInput").ap()
             for nm in ("lq1", "lq2", "lk1", "lk2")}
    rms_d = nc.dram_tensor("rmsw", [128, 1], F32, kind="ExternalInput").ap()
    y_d = nc.dram_tensor("y", [T, C], F32, kind="ExternalOutput").ap()

    with tile.TileContext(nc) as tc, ExitStack() as ctx:
        const = ctx.enter_context(tc.tile_pool(name="const", bufs=1))
        wpool = ctx.enter_context(tc.tile_pool(name="wpool", bufs=1))
        stage = ctx.enter_context(tc.tile_pool(name="stage", bufs=2))
        bstage = ctx.enter_context(tc.tile_pool(name="bstage", bufs=3))
        xch = ctx.enter_context(tc.tile_pool(name="xch", bufs=2))
        persist = ctx.enter_context(tc.tile_pool(name="persist", bufs=1))
        apool = ctx.enter_context(tc.tile_pool(name="apool", bufs=38))
        small = ctx.enter_context(tc.tile_pool(name="small", bufs=2))
        bcast = ctx.enter_context(tc.tile_pool(name="bcast", bufs=4))
        work = ctx.enter_context(tc.tile_pool(name="work", bufs=2))
        oscp = ctx.enter_context(tc.tile_pool(name="oscp", bufs=3))
        yout = ctx.enter_context(tc.tile_pool(name="yout", bufs=2))
        ps_a = ctx.enter_context(tc.tile_pool(name="ps_a", bufs=2, space="PSUM"))
        ps_p = ctx.enter_context(tc.tile_pool(name="ps_p", bufs=1, space="PSUM"))
        ps_o = ctx.enter_context(tc.tile_pool(name="ps_o", bufs=2, space="PSUM"))
        ps_s = ctx.enter_context(tc.tile_pool(name="ps_s", bufs=2, space="PSUM"))
        ps_y = ctx.enter_context(tc.tile_pool(name="ps_y", bufs=1, space="PSUM"))

        # ---------------- constants ----------------
        # causal mask applied on PE: scores += (-30000*I)^T @ strict-lower-tri
        negI = const.tile([128, 128], BF16, tag="negI")
        make_identity(nc, negI[:])
        nc.vector.tensor_scalar_mul(negI[:], negI[:], -30000.0)
        ltm = const.tile([128, 128], BF16, tag="ltm")
        nc.gpsimd.memset(ltm[:], 1.0)
        nc.gpsimd.affine_select(
            out=ltm[:], in_=ltm[:], compare_op=mybir.AluOpType.is_gt,
            fill=0.0, base=0, pattern=[[-1, 128]], channel_multiplier=1)
        ones_bf = const.tile([128, 1], BF16, tag="ones")
        nc.vector.memset(ones_bf[:], 1.0)
        ones_row = const.tile([1, 128], F32, tag="ones_row")
        nc.vector.memset(ones_row[:], 1.0)

        lamt = {}
        for nm in ("lq1", "lq2", "lk1", "lk2"):
            t_ = const.tile([1, HHD], F32, tag=nm)
            nc.sync.dma_start(t_[:], lam_d[nm])
            lamt[nm] = t_
        evals = []
        for a, b in (("lq1", "lk1"), ("lq2", "lk2")):
            m_ = small.tile([1, HHD], F32, tag="lmul")
            nc.vector.tensor_mul(m_[:], lamt[a][:], lamt[b][:])
            s_ = small.tile([1, 1], F32, tag="lsum")
            nc.vector.tensor_reduce(s_[:], m_[:], axis=mybir.AxisListType.X,
                                    op=mybir.AluOpType.add)
            e_ = const.tile([1, 1], F32, tag=f"e_{a}")
            nc.scalar.activation(e_[:], s_[:], AF.Exp)
            evals.append(e_)
        # neglam = -(exp(s1) - exp(s2) + LAMBDA_INIT) = e2 - e1 - LAMBDA_INIT
        neglam = const.tile([1, 1], F32, tag="neglam")
        nc.vector.tensor_sub(neglam[:], evals[1][:], evals[0][:])
        nc.vector.tensor_scalar_add(neglam[:], neglam[:], -LAMBDA_INIT)
        neglam128 = const.tile([128, 1], F32, tag="neglam128")

        eps_t = const.tile([1, 1], F32, tag="eps")
        nc.vector.memset(eps_t[:], 1e-5)
        rms_t = const.tile([128, 1], F32, tag="rms")
        nc.sync.dma_start(rms_t[:], rms_d)
        woscale = const.tile([128, 1], F32, tag="wos")
        nc.vector.tensor_scalar_mul(woscale[:], rms_t[:], 1.0 - LAMBDA_INIT)

        # ---------------- weights ----------------
        wq_bf = wpool.tile([128, 8, 256], BF16, tag="wq")
        wk_bf = wpool.tile([128, 8, 256], BF16, tag="wk")
        wv_bf = wpool.tile([128, 8, 256], BF16, tag="wv")
        for wd, wt in ((wv_d, wv_bf), (wk_d, wk_bf), (wq_d, wq_bf)):
            for g2 in range(4):
                ws = stage.tile([128, 2, 256], F32, tag="wstg", bufs=4)
                nc.sync.dma_start(
                    ws[:], wd[256 * g2:256 * (g2 + 1), :].rearrange(
                        "(r p) c -> p r c", p=128))
                nc.vector.tensor_copy(wt[:, 2 * g2:2 * (g2 + 1), :], ws[:])
        wo_bf = wpool.tile([128, 2, C], BF16, tag="wo")
        for h in range(2):
            ws = stage.tile([128, C], F32, tag="stg")
            nc.sync.dma_start(ws[:], wo_d[128 * h:128 * (h + 1), :])
            nc.vector.tensor_scalar_mul(ws[:], ws[:], woscale[:])
            nc.vector.tensor_copy(wo_bf[:, h, :], ws[:])

        # ---- fused per-chunk pipeline: loads/projections j overlap attention ----
        rep_ctx = tc.For_i(0, repeats, 1) if repeats > 1 else None
        if rep_ctx is not None:
            rep_ctx.__enter__()
        V_all = persist.tile([128, NT, 256], BF16, tag="V_all")
        kT_h = [persist.tile([128, T], BF16, tag=f"kT{h}", name=f"kT{h}") for h in range(2)]
        qT_h = [persist.tile([128, T], BF16, tag=f"qT{h}", name=f"qT{h}") for h in range(2)]

        def emit_loads(j):
            # value rows 512j..512(j+1) -> V_all n-tiles 4j..4j+3
            bstv = bstage.tile([128, 4, C], BF16, tag="bstg", name="bstv")
            nc.gpsimd.dma_start(
                bstv[:], v_d[TCH * j:TCH * (j + 1), :].rearrange(
                    "(r p) c -> p r c", p=128))
            for ii in range(4):
                i = 4 * j + ii
                xv = xch.tile([128, 8, 128], BF16, tag="xsm", bufs=6, name="xv")
                nc.sync.dma_start(xv[:], bstv[:, ii, :], transpose=True)
                pv = ps_p.tile([128, 512], F32, tag="pp", name="pv")
                for g in range(8):
                    nc.tensor.matmul(pv[:, :256], xv[:, g, :], wv_bf[:, g, :],
                                     start=(g == 0), stop=(g == 7))
                nc.vector.tensor_copy(V_all[:, i, :], pv[:, :256])
            # key chunk j -> kT_h[:, 512j:512(j+1)]
            xk = xch.tile([128, 8, TCH], BF16, tag="xbig", bufs=3, name="xk")
            bstk = bstage.tile([128, 4, C], BF16, tag="bstg", name="bstk")
            nc.gpsimd.dma_start(
                bstk[:], k_d[TCH * j:TCH * (j + 1), :].rearrange(
                    "(r p) c -> p r c", p=128))
            for kk in range(4):
                nc.sync.dma_start(xk[:, :, 128 * kk:128 * (kk + 1)],
                                  bstk[:, kk, :], transpose=True)
            for ds in range(2):
                pk = ps_p.tile([128, 512], F32, tag="pp", name="pk")
                for g in range(8):
                    nc.tensor.matmul(pk[:], wk_bf[:, g, 128 * ds:128 * (ds + 1)],
                                     xk[:, g, :], start=(g == 0), stop=(g == 7))
                nc.vector.tensor_copy(kT_h[ds][:, TCH * j:TCH * (j + 1)], pk[:])
            # query chunk j -> qT_h[:, 512j:512(j+1)]
            xq = xch.tile([128, 8, TCH], BF16, tag="xbig", bufs=3, name="xq")
            bstq = bstage.tile([128, 4, C], BF16, tag="bstg", name="bstq")
            nc.gpsimd.dma_start(
                bstq[:], q_d[TCH * j:TCH * (j + 1), :].rearrange(
                    "(r p) c -> p r c", p=128))
            for kk in range(4):
                nc.sync.dma_start(xq[:, :, 128 * kk:128 * (kk + 1)],
                                  bstq[:, kk, :], transpose=True)
            for ds in range(2):
                pq = ps_p.tile([128, 512], F32, tag="pp", name="pq")
                for g in range(8):
                    nc.tensor.matmul(pq[:], wq_bf[:, g, 128 * ds:128 * (ds + 1)],
                                     xq[:, g, :], start=(g == 0), stop=(g == 7))
                nc.vector.tensor_copy(qT_h[ds][:, TCH * j:TCH * (j + 1)], pq[:])

        for j in range(NCH):
            if j == 0:
                emit_loads(0)
            n_hi = 4 * j + 4
            # ---- phase A: per-head matmul blocks (scores/exp/O/rowsum) ----
            o_pss = []
            r_pss = []
            for h in range(2):
                A = {}
                for i in range(n_hi):
                    ts0 = max(0, 128 * i - TCH * j)
                    for hh in range(2):
                        r0 = 64 * hh
                        diag = i >= 4 * j
                        sp = ps_a.tile([128, 512], F32, tag="sps")
                        nc.tensor.matmul(
                            sp[:, ts0:], kT_h[h][r0:r0 + 64, 128 * i:128 * (i + 1)],
                            qT_h[h][r0:r0 + 64, TCH * j + ts0:TCH * (j + 1)],
                            start=True, stop=not diag)
                        if diag:
                            nc.tensor.matmul(sp[:, ts0:ts0 + 128], negI[:],
                                             ltm[:], start=False, stop=True)
                        at = apool.tile([128, 512], BF16, tag="at")
                        nc.scalar.activation(at[:, ts0:], sp[:, ts0:], AF.Exp,
                                             scale=SCALE)
                        A[(hh, i)] = (at, ts0)
                o_ps = [ps_o.tile([128, 512], F32, tag="ops", name=f"ops{_h}")
                        for _h in range(2)]
                for i in range(n_hi):
                    for hh in range(2):
                        at, ts0 = A[(hh, i)]
                        nc.tensor.matmul(o_ps[hh][:, ts0:],
                                         V_all[:, i, 128 * h:128 * (h + 1)],
                                         at[:, ts0:], start=(i == 0),
                                         stop=(i == n_hi - 1))
                # rowsums rows 0/32 = hh0/hh1; row 64 = rms square-sum
                r_ps = ps_s.tile([65, 512], F32, tag="rps")
                for i in range(n_hi):
                    for hh in range(2):
                        at, ts0 = A[(hh, i)]
                        ro = 32 * hh
                        nc.tensor.matmul(r_ps[ro:ro + 1, ts0:], ones_bf[:],
                                         at[:, ts0:], start=(i == 0),
                                         stop=(i == n_hi - 1))
                o_pss.append(o_ps)
                r_pss.append(r_ps)
            if j + 1 < NCH:
                emit_loads(j + 1)
            # ---- phase B: per-head softmax-division/combine/rms tails ----
            osc = []
            ocs = []
            for h in range(2):
                if j == 0 and h == 0:
                    nc.gpsimd.partition_broadcast(neglam128[:], neglam[:])
                o_ps, r_ps = o_pss[h], r_pss[h]
                rv1 = small.tile([1, 512], F32, tag="rv1")
                nc.vector.reciprocal_approx_fast(rv1[:], r_ps[0:1, :])
                rv2 = small.tile([1, 512], F32, tag="rv2")
                nc.vector.reciprocal_approx_fast(rv2[:], r_ps[32:33, :])
                bp1 = ps_y.tile([128, 512], F32, tag="ypb")
                nc.tensor.matmul(bp1[:], ones_row[:], rv1[:])
                B1 = bcast.tile([128, 512], F32, tag="B")
                nc.vector.tensor_copy(B1[:], bp1[:])
                bp2 = ps_y.tile([128, 512], F32, tag="ypb")
                nc.tensor.matmul(bp2[:], ones_row[:], rv2[:])
                B2 = bcast.tile([128, 512], F32, tag="B")
                nc.vector.tensor_copy(B2[:], bp2[:])
                t1 = work.tile([128, 512], F32, tag="wk1")
                nc.vector.tensor_mul(t1[:], o_ps[0][:], B1[:])
                t2 = work.tile([128, 512], F32, tag="wk2")
                nc.vector.scalar_tensor_tensor(
                    t2[:], o_ps[1][:], neglam128[:], B2[:],
                    op0=mybir.AluOpType.mult, op1=mybir.AluOpType.mult)
                oc = work.tile([128, 512], F32, tag="oc")
                nc.vector.tensor_add(oc[:], t1[:], t2[:])
                osq = work.tile([128, 512], BF16, tag="osq")
                nc.vector.tensor_mul(osq[:], oc[:], oc[:])
                nc.tensor.matmul(r_ps[64:65, :], ones_bf[:], osq[:])
                ocs.append(oc)
            for h in range(2):
                sq = small.tile([1, 512], F32, tag="sq")
                nc.scalar.activation(sq[:], r_pss[h][64:65, :], AF.Sqrt,
                                     bias=eps_t[:], scale=1.0 / DH)
                sv = small.tile([1, 512], F32, tag="sv")
                nc.vector.reciprocal_approx_fast(sv[:], sq[:])
                bp3 = ps_y.tile([128, 512], F32, tag="ypb")
                nc.tensor.matmul(bp3[:], ones_row[:], sv[:])
                SB = bcast.tile([128, 512], F32, tag="B")
                nc.vector.tensor_copy(SB[:], bp3[:])
                os_t = oscp.tile([128, 512], BF16, tag="osc")
                nc.vector.tensor_mul(os_t[:], ocs[h][:], SB[:])
                osc.append(os_t)
            # output projection: Y[t, :] = sum_h O_h^T(t)^T @ Wo_h
            for k4 in range(4):
                ysb = yout.tile([128, C], F32, tag="ysb")
                for half in range(2):
                    yp = ps_y.tile([128, 512], F32, tag="ypb")
                    for h in range(2):
                        lt = osc[h][:, 128 * k4:128 * (k4 + 1)]
                        nc.tensor.matmul(yp[:], lt,
                                         wo_bf[:, h, 512 * half:512 * (half + 1)],
                                         start=(h == 0), stop=(h == 1))
                    eng = nc.vector if (k4 + half) % 2 else nc.scalar
                    if eng is nc.vector:
                        nc.vector.tensor_copy(
                            ysb[:, 512 * half:512 * (half + 1)], yp[:])
                    else:
                        nc.scalar.copy(ysb[:, 512 * half:512 * (half + 1)], yp[:])
                row = TCH * j + 128 * k4
                nc.sync.dma_start(y_d[row:row + 128, :], ysb[:])
        if rep_ctx is not None:
            rep_ctx.__exit__(None, None, None)

    nc.compile()
    return nc


_NC_CACHE = {}


def get_nc(repeats=1):
    if repeats not in _NC_CACHE:
        _NC_CACHE[repeats] = build_nc(repeats)
    return _NC_CACHE[repeats]


def make_in_maps(query, key_t, value, Wq, Wk, Wv, Wo,
                 lambda_q1, lambda_q2, lambda_k1, lambda_k2, rms_weight):
    f = np.float32
    in_maps = []
    for core in range(8):
        b, p = divmod(core, 4)
        sl = slice(256 * p, 256 * (p + 1))
        in_maps.append({
            "query": np.ascontiguousarray(query[b], dtype=f),
            "key": np.ascontiguousarray(key_t[b], dtype=f),
            "value": np.ascontiguousarray(value[b], dtype=f),
            "wq": np.ascontiguousarray(Wq[:, sl], dtype=f),
            "wk": np.ascontiguousarray(Wk[:, sl], dtype=f),
            "wv": np.ascontiguousarray(Wv[:, sl], dtype=f),
            "wo": np.ascontiguousarray(Wo[sl, :], dtype=f),
            "lq1": np.ascontiguousarray(lambda_q1, dtype=f).reshape(1, HHD),
            "lq2": np.ascontiguousarray(lambda_q2, dtype=f).reshape(1, HHD),
            "lk1": np.ascontiguousarray(lambda_k1, dtype=f).reshape(1, HHD),
            "lk2": np.ascontiguousarray(lambda_k2, dtype=f).reshape(1, HHD),
            "rmsw": np.ascontiguousarray(rms_weight, dtype=f).reshape(128, 1),
        })
    return in_maps


def kernel(query, key_t, value, Wq, Wk, Wv, Wo,
           lambda_q1, lambda_q2, lambda_k1, lambda_k2, rms_weight):
    in_maps = make_in_maps(query, key_t, value, Wq, Wk, Wv, Wo,
                           lambda_q1, lambda_q2, lambda_k1, lambda_k2,
                           rms_weight)
    res = run_bass_kernel_spmd(get_nc(), in_maps, core_ids=list(range(8)))
    parts = np.stack([res.results[i]["y"] for i in range(8)])
    return parts.reshape(2, 4, T, C).sum(axis=1).astype(np.float32)


def build_nc_baseline():
    """Same I/O signature, near-empty body — for dispatch-overhead timing."""
    nc = bacc.Bacc("TRN2", target_bir_lowering=False, debug=False,
                   enable_asserts=False)
    nc.dram_tensor("query", [T, C], F32, kind="ExternalInput")
    nc.dram_tensor("key", [T, C], F32, kind="ExternalInput")
    nc.dram_tensor("value", [T, C], F32, kind="ExternalInput")
    nc.dram_tensor("wq", [C, 256], F32, kind="ExternalInput")
    nc.dram_tensor("wk", [C, 256], F32, kind="ExternalInput")
    nc.dram_tensor("wv", [C, 256], F32, kind="ExternalInput")
    nc.dram_tensor("wo", [256, C], F32, kind="ExternalInput")
    for nm in ("lq1", "lq2", "lk1", "lk2"):
        nc.dram_tensor(nm, [1, HHD], F32, kind="ExternalInput")
    nc.dram_tensor("rmsw", [128, 1], F32, kind="ExternalInput")
    y_d = nc.dram_tensor("y", [T, C], F32, kind="ExternalOutput").ap()
    with tile.TileContext(nc) as tc, ExitStack() as ctx:
        pool = ctx.enter_context(tc.tile_pool(name="p", bufs=1))
        t_ = pool.tile([128, C], F32, tag="t")
        nc.vector.memset(t_[:], 0.0)
        nc.sync.dma_start(y_d[0:128, :], t_[:])
    nc.compile()
    return nc
